# revision 22
# baseline (speedup 1.0000x reference)
"""Trainium2 Bass kernel for a CLIP encoder layer (B=32, S=257, E=1024, H=16, I=4096).

Strategy: data-parallel over batch across 8 NeuronCores (4 batch elements per
core), no collectives.  Per-core compute is feature-major ([E, tokens]) so
projection matmuls need no on-device transposes.

v2: Q/K/V/out-proj and fc1 run in fp8(e4m3) with DoubleRow perf mode (two
128-feature chunks contracted per PE pass).  Weights are scaled by a power of
two into the e4m3 range on the host; the de-scale is folded into cheap spots:
  - q/k: scores are computed on scaled q/k, exp() applies 1/(s_q*s_k) via the
    ACT scale input.
  - v: the PSUM->SBUF copy multiplies by 1/s_v.
  - out-proj: the residual-add STT multiplies by 1/s_o; o_b is folded into the
    residual x^T on the host.
  - fc1: gelu's ACT scale input applies 1/s_f1.
fc2 stays bf16 (fp8 there pushes rel-err past the 2e-2 gate).

Other structure:
  - LayerNorm column stats via PE ones-matmuls; sum and sum-of-squares run
    concurrently in different PE column groups (tile_position).  Squares are
    computed on GpSimd.  LN scale/bias folded into downstream weights host-side.
  - LN outputs written as fp8 into [128, KC, 1040] feature-chunk-major tiles so
    DoubleRow matmuls can take chunk-pair APs directly.
  - Attention: scores computed transposed (scores^T[j, i]) so softmax reduces
    over the partition dim via the ctx matmul; V tiles carry a ones column so
    the softmax denominators fall out of the ctx matmul for free; two heads
    (D=64) packed per PE pass at partition bases 0/64.
  - fc2: k-outer loop with balanced 343/343/342-column chunks so each bf16
    weight load serves ~3 matmuls.
"""

import numpy as np
import ml_dtypes

B, S, E, H, D, II = 32, 257, 1024, 16, 64, 4096
N_CORES = 8
B_LOC = B // N_CORES          # 4
NT = B_LOC * S                # 1028
NTP = 1040                    # padded NT (chunk stride, 16B aligned)
KC = E // 128                 # 8
MC_E = E // 128               # 8
MC_I = II // 128              # 32
EPS = 1e-5
F8MAX = 240.0

# balanced column chunks of NT for batch-agnostic bf16 matmuls (fc2)
CH3 = [(0, 343), (343, 343), (686, 342)]
# 512-wide chunks + 4-col tail for fp8 DoubleRow matmuls (DR LDWEIGHTS is
# 256 cols @1.2GHz ~ 213ns, hidden only when the matmul streams >=512 cols)
CH2 = [(0, 512), (512, 512)]
TAIL0, TAILN = 1024, 4
# j-chunks of one batch element's 257 keys
JC = [(0, 128), (128, 128), (256, 1)]

TRACE = False
LAST_EXEC_NS = None

_cache = {}


def _build(with_mask: bool, with_vbias: bool, with_qkbias: bool,
           sv_inv: float, so_inv: float, sqk_inv: float, sf1_inv: float):
    import concourse.tile as tile
    from concourse import bacc, mybir
    from contextlib import ExitStack

    F32 = mybir.dt.float32
    BF16 = mybir.dt.bfloat16
    FP8 = mybir.dt.float8e4
    AF = mybir.ActivationFunctionType
    ALU = mybir.AluOpType
    DR = mybir.MatmulPerfMode.DoubleRow

    nc = bacc.Bacc("TRN2", target_bir_lowering=False, debug=False,
                   enable_asserts=False, num_devices=N_CORES)

    xT_d = nc.dram_tensor("xT", [E, NT], F32, kind="ExternalInput")
    xTb_d = nc.dram_tensor("xTb", [E, NT], BF16, kind="ExternalInput")
    qw_d = nc.dram_tensor("qw", [MC_E, 128, KC, 128], FP8, kind="ExternalInput")
    kw_d = nc.dram_tensor("kw", [MC_E, 128, KC, 128], FP8, kind="ExternalInput")
    vw_d = nc.dram_tensor("vw", [KC // 2, 128, 2, E], FP8, kind="ExternalInput")
    ow_d = nc.dram_tensor("ow", [MC_E, 128, KC, 128], FP8, kind="ExternalInput")
    f1w_d = nc.dram_tensor("f1w", [MC_I, 128, KC, 128], FP8, kind="ExternalInput")
    f2w_d = nc.dram_tensor("f2w", [MC_E, 128, MC_I, 128], BF16, kind="ExternalInput")
    qb_d = nc.dram_tensor("qb", [128, MC_E], F32, kind="ExternalInput")
    kb_d = nc.dram_tensor("kb", [128, MC_E], F32, kind="ExternalInput")
    vb_d = nc.dram_tensor("vb", [1, E], F32, kind="ExternalInput")
    f1b_d = nc.dram_tensor("f1b", [128, MC_I], F32, kind="ExternalInput")
    f2b_d = nc.dram_tensor("f2b", [128, MC_E], F32, kind="ExternalInput")
    mskT_d = None
    if with_mask:
        mskT_d = nc.dram_tensor("mskT", [B_LOC, S, S], F32, kind="ExternalInput")
    outT_d = nc.dram_tensor("outT", [E, NT], F32, kind="ExternalOutput")

    with tile.TileContext(nc) as tc, ExitStack() as top:
        consts = top.enter_context(tc.tile_pool(name="consts", bufs=1))

        ones_col = consts.tile([128, 1], BF16)
        nc.vector.memset(ones_col[:], 1.0)
        ones_row = consts.tile([1, 128], BF16)
        nc.vector.memset(ones_row[:], 1.0)
        eps_t = consts.tile([1, 1], F32)
        nc.vector.memset(eps_t[:], EPS)
        qb_sb = consts.tile([128, MC_E], F32)
        nc.sync.dma_start(out=qb_sb[:], in_=qb_d[:])
        kb_sb = consts.tile([128, MC_E], F32)
        nc.sync.dma_start(out=kb_sb[:], in_=kb_d[:])
        f2b_sb = consts.tile([128, MC_E], F32)
        nc.sync.dma_start(out=f2b_sb[:], in_=f2b_d[:])
        f1b_sb = consts.tile([128, MC_I], F32)
        nc.sync.dma_start(out=f1b_sb[:], in_=f1b_d[:])
        vb_sb = consts.tile([128, E], F32)
        nc.sync.dma_start(out=vb_sb[:], in_=vb_d[0:1, :].to_broadcast((128, E)))

        def emit_ln(ph, src_ap, srcbf_ap, x_out, sfx, sq_pre=None,
                    split_apply=False):
            """Per-batch column LayerNorm over the feature (partition) dim.
            src_ap(k, b) -> [128, S] AP (f32 or bf16); srcbf_ap(k, b) ->
            [128, S] bf16 AP.  Writes (x - mu) * rstd as fp8 into
            x_out[:, k, b*S:(b+1)*S] (LN scale/bias folded into downstream
            weights host-side).  Sum and sum-of-squares matmuls run in PE
            column groups 0 / 1 concurrently.  sq_pre: optional dict of
            pre-computed square tiles keyed (k, b).  split_apply: run half
            the normalization ops on GpSimd (via SBUF copies of the
            broadcast rows) so fc1 can start sooner."""
            lntmp = ph.enter_context(tc.tile_pool(name=f"lntmp{sfx}", bufs=3))
            sqp = ph.enter_context(tc.tile_pool(name=f"sqp{sfx}", bufs=6))
            rows = ph.enter_context(tc.tile_pool(name=f"rows{sfx}", bufs=8))
            if split_apply:
                bcs = ph.enter_context(tc.tile_pool(name=f"bcs{sfx}", bufs=4))
            pstat = ph.enter_context(
                tc.tile_pool(name=f"pstat{sfx}", bufs=2, space="PSUM"))
            pbc = ph.enter_context(
                tc.tile_pool(name=f"pbc{sfx}", bufs=2, space="PSUM"))
            sq_engines = [nc.gpsimd.tensor_mul, nc.vector.tensor_mul]
            for b in range(B_LOC):
                ps = pstat.tile([128, 512], F32, name="ps_stat", tag="stat")
                sqs = []
                for k in range(KC):
                    xb = srcbf_ap(k, b)
                    # the sum chain runs on PE while squares compute
                    nc.tensor.matmul(ps[0:1, 0:S], ones_col[:], xb,
                                     start=(k == 0), stop=(k == KC - 1),
                                     tile_position=(0, 0))
                    if sq_pre is not None:
                        sqs.append(sq_pre[(k, b)])
                    else:
                        sq = sqp.tile([128, S], BF16, name="sq", tag="sq")
                        sq_engines[k % 2](out=sq[:], in0=xb, in1=xb)
                        sqs.append(sq)
                for k in range(KC):
                    nc.tensor.matmul(ps[32:33, 0:S], ones_col[:], sqs[k][:],
                                     start=(k == 0), stop=(k == KC - 1),
                                     tile_position=(0, 32))
                musq = rows.tile([1, S], F32, name="musq", tag="row")
                nc.scalar.activation(out=musq[0:1, :], in_=ps[0:1, 0:S],
                                     func=AF.Square, scale=-1.0 / E)
                muneg_b = rows.tile([1, S], BF16, name="muneg_b", tag="row")
                nc.scalar.mul(out=muneg_b[0:1, :], in_=ps[0:1, 0:S],
                              mul=-1.0 / E)
                var = rows.tile([1, S], F32, name="var", tag="row")
                nc.vector.scalar_tensor_tensor(
                    out=var[0:1, :], in0=ps[32:33, 0:S], scalar=1.0 / E,
                    in1=musq[0:1, :], op0=ALU.mult, op1=ALU.subtract)
                sd = rows.tile([1, S], F32, name="sd", tag="row")
                nc.scalar.activation(out=sd[0:1, :], in_=var[0:1, :],
                                     func=AF.Sqrt, bias=eps_t[0:1, 0:1])
                rstd = rows.tile([1, S], F32, name="rstd", tag="row")
                nc.vector.reciprocal_approx_fast(out=rstd[0:1, :],
                                                 in_=sd[0:1, :])
                rstd_b = rows.tile([1, S], BF16, name="rstd_b", tag="row")
                nc.vector.tensor_copy(out=rstd_b[0:1, :], in_=rstd[0:1, :])
                psA = pbc.tile([128, 512], F32, name="psA", tag="bc")
                psB = pbc.tile([128, 512], F32, name="psB", tag="bc")
                nc.tensor.matmul(psA[:, 0:S], ones_row[0:1, :],
                                 rstd_b[0:1, :], start=True, stop=True)
                nc.tensor.matmul(psB[:, 0:S], ones_row[0:1, :],
                                 muneg_b[0:1, :], start=True, stop=True)
                if split_apply and b >= 2:
                    # GpSimd cannot read PSUM: stage the broadcast rows in
                    # SBUF, then normalize on GpSimd to keep DVE free
                    sbA = bcs.tile([128, S], F32, name="sbA", tag="bcs")
                    nc.scalar.copy(out=sbA[:], in_=psA[:, 0:S])
                    sbB = bcs.tile([128, S], F32, name="sbB", tag="bcs")
                    nc.scalar.copy(out=sbB[:], in_=psB[:, 0:S])
                    for k in range(KC):
                        tmp = lntmp.tile([128, S], F32, name="tmp", tag="ap")
                        nc.gpsimd.tensor_add(out=tmp[:], in0=src_ap(k, b),
                                             in1=sbB[:])
                        nc.gpsimd.tensor_mul(
                            out=x_out[:, k, b * S:(b + 1) * S],
                            in0=tmp[:], in1=sbA[:])
                else:
                    for k in range(KC):
                        tmp = lntmp.tile([128, S], F32, name="tmp", tag="ap")
                        nc.vector.tensor_add(out=tmp[:], in0=src_ap(k, b),
                                             in1=psB[:, 0:S])
                        nc.vector.tensor_mul(
                            out=x_out[:, k, b * S:(b + 1) * S],
                            in0=tmp[:], in1=psA[:, 0:S])

        # fp8 chunk-major activations: [128, KC, NTP]
        x1_p = top.enter_context(tc.tile_pool(name="x1", bufs=1))
        x1 = x1_p.tile([128, KC, NTP], FP8, name="x1", tag="x1")
        x2_p = top.enter_context(tc.tile_pool(name="x2", bufs=1))
        x2 = x2_p.tile([128, KC, NTP], FP8, name="x2", tag="x2")

        # residual (x + o_b)^T f32: emitted on the Scalar engine's DMA
        # queue so weight DMAs (Sync queue) are not stuck behind these
        # 4.2MB; loads overlap LN1/V/attention.
        xt_p = top.enter_context(tc.tile_pool(name="xt", bufs=KC))
        xt = []
        for k in range(KC):
            t = xt_p.tile([128, NT], F32, name="xt", tag="xt")
            nc.scalar.dma_start(
                out=t[:], in_=xT_d[k * 128:(k + 1) * 128, :])
            xt.append(t)

        # long-lived right-side pools, opened in close order (LIFO):
        ht_p = top.enter_context(
            tc.tile_pool(name="ht", bufs=KC, side="right"))
        # closed explicitly after LN2 (holds bf16 copies + squares)
        ln2prep_ph = ExitStack()
        htb_p = ln2prep_ph.enter_context(
            tc.tile_pool(name="htb", bufs=KC, side="right"))
        sq2_p = ln2prep_ph.enter_context(
            tc.tile_pool(name="sq2", bufs=KC * B_LOC, side="right"))

        with tc.tile_pool(name="ctx2", bufs=MC_E // 2) as ctx_p:
            ctx2 = [ctx_p.tile([128, 2, NTP], FP8, tag="ctx2", name="ctx2")
                    for _ in range(MC_E // 2)]

            # ============= LN1 / V / QK+attention ====================
            with tc.tile_pool(name="vpool", bufs=2 * B_LOC + 1) as v_p:
                with ExitStack() as ln1_ph:
                    xtb_p = ln1_ph.enter_context(
                        tc.tile_pool(name="xtb", bufs=KC * B_LOC))
                    xtb = {}
                    for b in range(B_LOC):
                        for k in range(KC):
                            tb = xtb_p.tile([128, S], BF16, name="xtb",
                                            tag="xtb")
                            nc.sync.dma_start(
                                out=tb[:],
                                in_=xTb_d[k * 128:(k + 1) * 128,
                                          b * S:(b + 1) * S])
                            xtb[(k, b)] = tb
                    emit_ln(ln1_ph,
                            lambda k, b: xtb[(k, b)][:],
                            lambda k, b: xtb[(k, b)][:],
                            x1, "1")

                    # ============= V projection (fp8 DoubleRow) ==========
                    v_tiles = {}
                    with ExitStack() as ph:
                        vw_p = ph.enter_context(
                            tc.tile_pool(name="vw", bufs=KC // 2))
                        ppv = ph.enter_context(
                            tc.tile_pool(name="ppv", bufs=2, space="PSUM"))
                        vw_sb = []
                        for kk in range(KC // 2):
                            vwk = vw_p.tile([128, 2, E], FP8, name="vwk",
                                            tag="vwk")
                            nc.sync.dma_start(out=vwk[:], in_=vw_d[kk, :, :, :])
                            vw_sb.append(vwk)
                        for b in range(B_LOC):
                            for jc, (j0, jcs) in enumerate(JC[:2]):
                                ps = ppv.tile([128, 2, 512], F32,
                                              name="vps", tag="vps")
                                for kk in range(KC // 2):
                                    for n in range(2):
                                        nc.tensor.matmul(
                                            ps[0:jcs, n, :],
                                            x1[:, 2 * kk:2 * kk + 2,
                                               b * S + j0:b * S + j0 + jcs],
                                            vw_sb[kk][:, :, n * 512:
                                                      (n + 1) * 512],
                                            start=(kk == 0),
                                            stop=(kk == KC // 2 - 1),
                                            perf_mode=DR)
                                # [tok, H, 128]: cols 0:64 ones, cols
                                # 64:128 V -> ctx matmul replicates the
                                # softmax sums across partitions 0:64.
                                vt = v_p.tile([128, H, 128], BF16,
                                              name="vt", tag="vt")
                                if with_vbias:
                                    nc.vector.scalar_tensor_tensor(
                                        out=vt[0:jcs, :, 64:128],
                                        in0=ps[0:jcs, :, :], scalar=sv_inv,
                                        in1=vb_sb[0:jcs, :],
                                        op0=ALU.mult, op1=ALU.add)
                                else:
                                    nc.scalar.mul(
                                        out=vt[0:jcs, :, 64:128],
                                        in_=ps[0:jcs, :, :], mul=sv_inv)
                                nc.gpsimd.memset(vt[:, :, 0:64], 1.0)
                                v_tiles[(b, jc)] = vt
                        # the 4 batches' tail token (j=256): pack the
                        # M=1 matmuls into column groups 0/32/64/96 so
                        # they run concurrently on the PE array.
                        ps = ppv.tile([128, 2, 512], F32,
                                      name="vps_t", tag="vps")
                        for b in range(B_LOC):
                            for n in range(2):
                                for k in range(KC):
                                    nc.tensor.matmul(
                                        ps[32 * b:32 * b + 1, n, :],
                                        x1[:, k, b * S + 256:b * S + 257],
                                        vw_sb[k // 2][:, k % 2,
                                                      n * 512:(n + 1) * 512],
                                        start=(k == 0), stop=(k == KC - 1),
                                        tile_position=(0, 32 * b))
                        vt_t = v_p.tile([128, H, 128], BF16,
                                        name="vt_t", tag="vt")
                        nc.gpsimd.memset(vt_t[:, :, 0:64], 1.0)
                        for b in range(B_LOC):
                            if with_vbias:
                                nc.vector.scalar_tensor_tensor(
                                    out=vt_t[32 * b:32 * b + 1, :, 64:128],
                                    in0=ps[32 * b:32 * b + 1, :, :],
                                    scalar=sv_inv, in1=vb_sb[0:1, :],
                                    op0=ALU.mult, op1=ALU.add)
                            else:
                                nc.scalar.mul(
                                    out=vt_t[32 * b:32 * b + 1, :, 64:128],
                                    in_=ps[32 * b:32 * b + 1, :, :],
                                    mul=sv_inv)
                            v_tiles[(b, 2)] = vt_t

                # ========= Q/K + attention (per head-pair chunk) =====
                with ExitStack() as ph:
                    qt_p = ph.enter_context(tc.tile_pool(name="qt", bufs=2))
                    kt_p = ph.enter_context(tc.tile_pool(name="kt", bufs=2))
                    wqk_p = ph.enter_context(
                        tc.tile_pool(name="wqk", bufs=4))
                    e_p = ph.enter_context(tc.tile_pool(name="ep", bufs=9))
                    rs_p = ph.enter_context(tc.tile_pool(name="rsp", bufs=4))
                    if with_mask:
                        msk_p = ph.enter_context(
                            tc.tile_pool(name="mskp", bufs=3 * B_LOC))
                    pp2 = ph.enter_context(
                        tc.tile_pool(name="pp2", bufs=1, space="PSUM"))
                    psp = ph.enter_context(
                        tc.tile_pool(name="psp", bufs=3, space="PSUM"))
                    if with_mask:
                        msk = {}
                        for b in range(B_LOC):
                            for jc, (j0, jcs) in enumerate(JC):
                                mt = msk_p.tile([128, S], F32, name="mt",
                                                tag="mt")
                                nc.sync.dma_start(
                                    out=mt[0:jcs, :],
                                    in_=mskT_d[b, j0:j0 + jcs, :])
                                msk[(b, jc)] = mt

                    for ec in range(MC_E):
                        # ps_t serves three roles this iteration: spare
                        # columns 504:508 hold the Q/K projection 4-col
                        # tails, then rows 32b get the tail-key scores.
                        ps_t = psp.tile([128, 2, 512], F32,
                                        name="ps_t", tag="sp")
                        nc.vector.memset(ps_t[:], 0.0)
                        qkt = []
                        for ip, (w_d, b_sb, opool) in enumerate((
                                (qw_d, qb_sb, qt_p),
                                (kw_d, kb_sb, kt_p))):
                            wt = wqk_p.tile([128, KC, 128], FP8,
                                            name="wqk", tag="wqk")
                            nc.sync.dma_start(out=wt[:],
                                              in_=w_d[ec, :, :, :])
                            ot = opool.tile([128, NT], BF16,
                                            name="qk", tag="qk")
                            ps = pp2.tile([128, 2, 512], F32,
                                          name="pqk", tag="pqk")
                            for c, (c0, cn) in enumerate(CH2):
                                for kk in range(KC // 2):
                                    nc.tensor.matmul(
                                        ps[:, c, 0:cn],
                                        wt[:, 2 * kk:2 * kk + 2, :],
                                        x1[:, 2 * kk:2 * kk + 2,
                                           c0:c0 + cn],
                                        start=(kk == 0),
                                        stop=(kk == KC // 2 - 1),
                                        perf_mode=DR)
                            # 4-col tail in normal fp8 mode (a DR tail
                            # matmul would pay a full 256-col LDWEIGHTS)
                            for k in range(KC):
                                nc.tensor.matmul(
                                    ps_t[:, ip, 504:504 + TAILN],
                                    wt[:, k, :],
                                    x1[:, k, TAIL0:TAIL0 + TAILN],
                                    start=(k == 0), stop=(k == KC - 1))
                            # drains split across ACT and DVE so the
                            # single pp2 buffer frees quickly
                            drains = [(ps[:, 0, 0:512], 0, 512),
                                      (ps[:, 1, 0:512], 512, 512),
                                      (ps_t[:, ip, 504:504 + TAILN],
                                       TAIL0, TAILN)]
                            for c, (src, c0, cn) in enumerate(drains):
                                if with_qkbias:
                                    nc.vector.tensor_scalar_add(
                                        out=ot[:, c0:c0 + cn], in0=src,
                                        scalar1=b_sb[:, ec:ec + 1])
                                elif (c + ip) % 2 == 0:
                                    nc.scalar.copy(out=ot[:, c0:c0 + cn],
                                                   in_=src)
                                else:
                                    nc.vector.tensor_copy(
                                        out=ot[:, c0:c0 + cn], in_=src)
                            qkt.append(ot)
                        qte, kte = qkt

                        # tail key (j=256) for all 4 batches: packed
                        # into array col groups 32b / row groups 64*hi,
                        # one shared exp over all rows.
                        for b in range(B_LOC):
                            for hi in range(2):
                                p0 = hi * 64
                                nc.tensor.matmul(
                                    ps_t[32 * b:32 * b + 1, hi, 0:S],
                                    kte[p0:p0 + 64,
                                        b * S + 256: b * S + 257],
                                    qte[p0:p0 + 64, b * S:(b + 1) * S],
                                    start=True, stop=True,
                                    tile_position=(p0, 32 * b))
                        if with_mask:
                            for b in range(B_LOC):
                                for hi in range(2):
                                    nc.vector.tensor_add(
                                        out=ps_t[32 * b:32 * b + 1, hi, 0:S],
                                        in0=ps_t[32 * b:32 * b + 1, hi, 0:S],
                                        in1=msk[(b, 2)][0:1, :])
                        et_t = e_p.tile([128, 2, S], BF16,
                                        name="et_t", tag="et")
                        nc.scalar.activation(out=et_t[:, :, :],
                                             in_=ps_t[:, :, 0:S],
                                             func=AF.Exp, scale=sqk_inv)

                        for b in range(B_LOC):
                            ets = []
                            for jc, (j0, jcs) in enumerate(JC[:2]):
                                sp = psp.tile([128, 2, 512], F32,
                                              name="sp", tag="sp")
                                for hi in range(2):
                                    p0 = hi * 64
                                    nc.tensor.matmul(
                                        sp[0:jcs, hi, 0:S],
                                        kte[p0:p0 + 64,
                                            b * S + j0: b * S + j0 + jcs],
                                        qte[p0:p0 + 64,
                                            b * S:(b + 1) * S],
                                        start=True, stop=True)
                                if with_mask:
                                    for hi in range(2):
                                        nc.vector.tensor_add(
                                            out=sp[0:jcs, hi, 0:S],
                                            in0=sp[0:jcs, hi, 0:S],
                                            in1=msk[(b, jc)][0:jcs, :])
                                et = e_p.tile([128, 2, S], BF16,
                                              name="et", tag="et")
                                nc.scalar.activation(
                                    out=et[0:jcs, :, :],
                                    in_=sp[0:jcs, :, 0:S], func=AF.Exp,
                                    scale=sqk_inv)
                                ets.append(et)
                            cp = psp.tile([128, 2, 512], F32,
                                          name="cp", tag="sp")
                            for hi in range(2):
                                h = 2 * ec + hi
                                for jc, (j0, jcs) in enumerate(JC[:2]):
                                    nc.tensor.matmul(
                                        cp[0:128, hi, 0:S],
                                        v_tiles[(b, jc)][0:jcs, h, :],
                                        ets[jc][0:jcs, hi, :],
                                        start=(jc == 0), stop=False)
                                nc.tensor.matmul(
                                    cp[0:128, hi, 0:S],
                                    v_tiles[(b, 2)][32 * b:32 * b + 1,
                                                    h, :],
                                    et_t[32 * b:32 * b + 1, hi, :],
                                    start=False, stop=True,
                                    tile_position=(32 * b, 0))
                            rst = rs_p.tile([64, 2, S], F32,
                                            name="rst", tag="rst")
                            nc.vector.reciprocal_approx_fast(
                                out=rst[0:64, :, :],
                                in_=cp[0:64, :, 0:S])
                            for hi in range(2):
                                nc.vector.tensor_mul(
                                    out=ctx2[ec // 2][hi * 64:hi * 64 + 64,
                                                      ec % 2,
                                                      b * S:(b + 1) * S],
                                    in0=cp[64:128, hi, 0:S],
                                    in1=rst[0:64, hi, :])

            # ===== out projection (fp8 DoubleRow) + residual =====
            # LN2 prep (bf16 copy + squares) is interleaved per m-tile so
            # the LN2 stats can start the moment the projection finishes.
            ht, htb, sq2 = [], [], {}
            with ExitStack() as ph:
                wo_p = ph.enter_context(tc.tile_pool(name="wo", bufs=3))
                ppo = ph.enter_context(
                    tc.tile_pool(name="ppo", bufs=2, space="PSUM"))
                for m in range(MC_E):
                    wt = wo_p.tile([128, KC, 128], FP8, name="wo",
                                   tag="wo")
                    nc.sync.dma_start(out=wt[:], in_=ow_d[m, :, :, :])
                    ps = ppo.tile([128, 3, 512], F32, name="po",
                                  tag="po")
                    for c, (c0, cn) in enumerate(CH2):
                        for kk in range(KC // 2):
                            nc.tensor.matmul(
                                ps[:, c, 0:cn],
                                wt[:, 2 * kk:2 * kk + 2, :],
                                ctx2[kk][:, :, c0:c0 + cn],
                                start=(kk == 0), stop=(kk == KC // 2 - 1),
                                perf_mode=DR)
                    for k in range(KC):
                        nc.tensor.matmul(
                            ps[:, 2, 0:TAILN], wt[:, k, :],
                            ctx2[k // 2][:, k % 2, TAIL0:TAIL0 + TAILN],
                            start=(k == 0), stop=(k == KC - 1))
                    o = ht_p.tile([128, NT], F32, name="ht", tag="ht")
                    for c, c0, cn in ((0, 0, 512), (1, 512, 512),
                                      (2, TAIL0, TAILN)):
                        nc.vector.scalar_tensor_tensor(
                            out=o[:, c0:c0 + cn],
                            in0=ps[:, c, 0:cn],
                            scalar=so_inv,
                            in1=xt[m][:, c0:c0 + cn],
                            op0=ALU.mult, op1=ALU.add)
                    ht.append(o)
                    hb = htb_p.tile([128, NT], BF16, name="htb", tag="htb")
                    nc.gpsimd.tensor_copy(out=hb[:], in_=o[:])
                    htb.append(hb)
                    for b in range(B_LOC):
                        sq = sq2_p.tile([128, S], BF16, name="sq2",
                                        tag="sq2")
                        if b % 2 == 0:
                            nc.scalar.activation(
                                out=sq[:], in_=hb[:, b * S:(b + 1) * S],
                                func=AF.Square)
                        else:
                            nc.vector.tensor_mul(
                                out=sq[:], in0=hb[:, b * S:(b + 1) * S],
                                in1=hb[:, b * S:(b + 1) * S])
                        sq2[(m, b)] = sq
        # ctx2 closed

        # ================= LN2 + MLP =====================================
        with ExitStack() as ln2_ph:
            emit_ln(ln2_ph,
                    lambda k, b: ht[k][:, b * S:(b + 1) * S],
                    lambda k, b: htb[k][:, b * S:(b + 1) * S],
                    x2, "2", sq_pre=sq2, split_apply=True)
        ln2prep_ph.close()
        # f1o reuses the SBUF freed by htb/sq2
        f1o_p = top.enter_context(
            tc.tile_pool(name="f1o", bufs=MC_I, side="right"))

        # ============= fc1 (fp8 DoubleRow) ==============================
        f1o = []
        with ExitStack() as ph:
            # fc2 weight pool opens early so the first fc2 weights prefetch
            # during fc1
            wf2_p = ph.enter_context(tc.tile_pool(name="wf2", bufs=2))
            f2w_t = {}
            with ExitStack() as ph1:
                wf1_p = ph1.enter_context(tc.tile_pool(name="wf1", bufs=4))
                ppf1 = ph1.enter_context(
                    tc.tile_pool(name="ppf1", bufs=2, space="PSUM"))
                for m in range(MC_I):
                    wt = wf1_p.tile([128, KC, 128], FP8, name="wf1",
                                    tag="wf1")
                    nc.sync.dma_start(out=wt[:], in_=f1w_d[m, :, :, :])
                    o = f1o_p.tile([128, NT], BF16, name="f1o", tag="f1o")
                    ps = ppf1.tile([128, 3, 512], F32, name="pf1", tag="pf1")
                    for c, (c0, cn) in enumerate(CH2):
                        for kk in range(KC // 2):
                            nc.tensor.matmul(
                                ps[:, c, 0:cn],
                                wt[:, 2 * kk:2 * kk + 2, :],
                                x2[:, 2 * kk:2 * kk + 2, c0:c0 + cn],
                                start=(kk == 0), stop=(kk == KC // 2 - 1),
                                perf_mode=DR)
                    for k in range(KC):
                        nc.tensor.matmul(
                            ps[:, 2, 0:TAILN], wt[:, k, :],
                            x2[:, k, TAIL0:TAIL0 + TAILN],
                            start=(k == 0), stop=(k == KC - 1))
                    for c, c0, cn in ((0, 0, 512), (1, 512, 512),
                                      (2, TAIL0, TAILN)):
                        nc.scalar.activation(
                            out=o[:, c0:c0 + cn],
                            in_=ps[:, c, 0:cn],
                            func=AF.Gelu_apprx_tanh, scale=sf1_inv,
                            bias=f1b_sb[:, m:m + 1])
                    f1o.append(o)
                    if m == MC_I // 2:
                        # kick off fc2 weight DMAs mid-fc1 so the fc2 loop
                        # starts hot
                        for mm in range(2):
                            wt2 = wf2_p.tile([128, MC_I, 128], BF16,
                                             name="wf2", tag="wf2")
                            nc.sync.dma_start(out=wt2[:],
                                              in_=f2w_d[mm, :, :, :])
                            f2w_t[mm] = wt2

            # ============= fc2 (bf16, k-outer, 343-col chunks) ==========
            with ExitStack() as ph2:
                ppf2 = ph2.enter_context(
                    tc.tile_pool(name="ppf2", bufs=2, space="PSUM"))
                out_p = ph2.enter_context(tc.tile_pool(name="outp", bufs=3))
                for m in range(MC_E):
                    if m not in f2w_t:
                        wt2 = wf2_p.tile([128, MC_I, 128], BF16,
                                         name="wf2", tag="wf2")
                        nc.sync.dma_start(out=wt2[:], in_=f2w_d[m, :, :, :])
                        f2w_t[m] = wt2
                    wt = f2w_t.pop(m)
                    ps = ppf2.tile([128, 3, 512], F32, name="pf2", tag="pf2")
                    for k in range(MC_I):
                        for c, (c0, cn) in enumerate(CH3):
                            nc.tensor.matmul(
                                ps[:, c, 0:cn], wt[:, k, :],
                                f1o[k][:, c0:c0 + cn],
                                start=(k == 0), stop=(k == MC_I - 1))
                    o = out_p.tile([128, NT], F32, name="oo", tag="oo")
                    for c, (c0, cn) in enumerate(CH3):
                        nc.vector.scalar_tensor_tensor(
                            out=o[:, c0:c0 + cn], in0=ps[:, c, 0:cn],
                            scalar=f2b_sb[:, m:m + 1],
                            in1=ht[m][:, c0:c0 + cn],
                            op0=ALU.add, op1=ALU.add)
                    nc.sync.dma_start(out=outT_d[m * 128:(m + 1) * 128, :],
                                      in_=o[:])

    nc.compile()
    return nc


def _pow2_scale(W):
    """Largest power-of-two s with max|W|*s <= F8MAX."""
    m = float(np.abs(W).max())
    if m == 0.0:
        return 1.0
    return 2.0 ** np.floor(np.log2(F8MAX / m))


def _pack_lhsT8(W, s):
    """W [M, K] (out, in) -> [M/128, 128, K/128, 128] fp8 with
    [m, p, k, j] = W[m*128+j, k*128+p] * s (lhsT tiles, partition = K)."""
    W = np.asarray(W, np.float32) * s
    M, K = W.shape
    A = W.reshape(M // 128, 128, K // 128, 128)
    A = np.ascontiguousarray(A.transpose(0, 3, 2, 1))
    return np.clip(A, -F8MAX, F8MAX).astype(ml_dtypes.float8_e4m3)


def _pack_lhsT(W):
    """W [M, K] -> [M/128, 128, K/128, 128] bf16 lhsT tiles."""
    W = np.asarray(W, np.float32)
    M, K = W.shape
    A = W.reshape(M // 128, 128, K // 128, 128)
    return np.ascontiguousarray(A.transpose(0, 3, 2, 1)).astype(ml_dtypes.bfloat16)


def _pack_pbias(b):
    """b [M] -> [128, M/128] f32 per-partition bias columns."""
    return np.ascontiguousarray(np.asarray(b, np.float32).reshape(-1, 128).T)


def kernel(hidden_states, attention_mask, causal_attention_mask,
           ln1_w, ln1_b, q_w, q_b, k_w, k_b, v_w, v_b, o_w, o_b,
           ln2_w, ln2_b, fc1_w, fc1_b, fc2_w, fc2_b):
    global LAST_EXEC_NS
    from concourse.bass_utils import run_bass_kernel_spmd

    hs = np.asarray(hidden_states, np.float32)
    msk = (np.asarray(attention_mask, np.float32)
           + np.asarray(causal_attention_mask, np.float32))
    with_mask = bool(np.any(msk))

    ln1_w = np.asarray(ln1_w, np.float32); ln1_b = np.asarray(ln1_b, np.float32)
    ln2_w = np.asarray(ln2_w, np.float32); ln2_b = np.asarray(ln2_b, np.float32)
    q_w = np.asarray(q_w, np.float32); q_b = np.asarray(q_b, np.float32)
    k_w = np.asarray(k_w, np.float32); k_b = np.asarray(k_b, np.float32)
    v_w = np.asarray(v_w, np.float32); v_b = np.asarray(v_b, np.float32)
    o_w = np.asarray(o_w, np.float32); o_b = np.asarray(o_b, np.float32)
    fc1_w = np.asarray(fc1_w, np.float32); fc1_b = np.asarray(fc1_b, np.float32)
    fc2_w = np.asarray(fc2_w, np.float32); fc2_b = np.asarray(fc2_b, np.float32)

    scale = D ** -0.5
    # fold LN1 scale/bias into Q/K/V, and the softmax scale into Q
    qw_eff = (q_w * ln1_w[None, :]) * scale
    qb_eff = (q_b + q_w @ ln1_b) * scale
    kw_eff = k_w * ln1_w[None, :]
    kb_eff = k_b + k_w @ ln1_b
    vw_eff = v_w * ln1_w[None, :]
    vb_eff = v_b + v_w @ ln1_b
    # fold LN2 into fc1
    f1w_eff = fc1_w * ln2_w[None, :]
    f1b_eff = fc1_b + fc1_w @ ln2_b

    # fp8 power-of-two scales
    s_q = _pow2_scale(qw_eff)
    s_k = _pow2_scale(kw_eff)
    s_v = _pow2_scale(vw_eff)
    s_o = _pow2_scale(o_w)
    s_f1 = _pow2_scale(f1w_eff)

    vw8 = np.clip(vw_eff.T * s_v, -F8MAX, F8MAX).astype(
        ml_dtypes.float8_e4m3).reshape(KC // 2, 2, 128, E).transpose(0, 2, 1, 3)

    base = {
        "qw": _pack_lhsT8(qw_eff, s_q),
        "kw": _pack_lhsT8(kw_eff, s_k),
        "vw": np.ascontiguousarray(vw8),
        "ow": _pack_lhsT8(o_w, s_o),
        "f1w": _pack_lhsT8(f1w_eff, s_f1),
        "f2w": _pack_lhsT(fc2_w),
        "qb": _pack_pbias(qb_eff * s_q),
        "kb": _pack_pbias(kb_eff * s_k),
        "vb": np.ascontiguousarray(vb_eff[None, :].astype(np.float32)),
        "f1b": _pack_pbias(f1b_eff),
        "f2b": _pack_pbias(fc2_b),
    }

    with_vbias = bool(np.any(vb_eff))
    with_qkbias = bool(np.any(qb_eff)) or bool(np.any(kb_eff))
    key = (with_mask, with_vbias, with_qkbias, s_q, s_k, s_v, s_o, s_f1)
    if key not in _cache:
        _cache[key] = _build(with_mask, with_vbias, with_qkbias,
                             1.0 / s_v, 1.0 / s_o, 1.0 / (s_q * s_k),
                             1.0 / s_f1)
    nc = _cache[key]

    # o_b folded into the residual
    res = hs + o_b[None, None, :]

    in_maps = []
    for c in range(N_CORES):
        x = hs[c * B_LOC:(c + 1) * B_LOC].reshape(NT, E).T
        r = res[c * B_LOC:(c + 1) * B_LOC].reshape(NT, E).T
        m = dict(base)
        m["xT"] = np.ascontiguousarray(r)
        m["xTb"] = np.ascontiguousarray(x).astype(ml_dtypes.bfloat16)
        if with_mask:
            m["mskT"] = np.ascontiguousarray(
                msk[c * B_LOC:(c + 1) * B_LOC, 0].transpose(0, 2, 1)
                * (s_q * s_k))
        in_maps.append(m)

    res_k = run_bass_kernel_spmd(nc, in_maps, core_ids=list(range(N_CORES)),
                                 trace=TRACE)
    LAST_EXEC_NS = res_k.exec_time_ns

    outs = []
    for c in range(N_CORES):
        oT = res_k.results[c]["outT"]          # [E, NT] f32
        outs.append(np.ascontiguousarray(oT.T).reshape(B_LOC, S, E))
    return np.concatenate(outs, axis=0)


# revision 29
# speedup vs baseline: 1.2028x; 1.2028x over previous
"""Trainium2 Bass kernel for a CLIP encoder layer (B=32, S=257, E=1024, H=16, I=4096).

Strategy: data-parallel over batch across 8 NeuronCores (4 batch elements per
core), no collectives.  Per-core compute is feature-major ([E, tokens]) so
projection matmuls need no on-device transposes.

v2: Q/K/V/out-proj and fc1 run in fp8(e4m3) with DoubleRow perf mode (two
128-feature chunks contracted per PE pass).  Weights are scaled by a power of
two into the e4m3 range on the host; the de-scale is folded into cheap spots:
  - q/k: scores are computed on scaled q/k, exp() applies 1/(s_q*s_k) via the
    ACT scale input.
  - v: the PSUM->SBUF copy multiplies by 1/s_v.
  - out-proj: the residual-add STT multiplies by 1/s_o; o_b is folded into the
    residual x^T on the host.
  - fc1: gelu's ACT scale input applies 1/s_f1.
fc2 stays bf16 (fp8 there pushes rel-err past the 2e-2 gate).

Other structure:
  - LayerNorm column stats via PE ones-matmuls; sum and sum-of-squares run
    concurrently in different PE column groups (tile_position).  Squares are
    computed on GpSimd.  LN scale/bias folded into downstream weights host-side.
  - LN outputs written as fp8 into [128, KC, 1040] feature-chunk-major tiles so
    DoubleRow matmuls can take chunk-pair APs directly.
  - Attention: scores computed transposed (scores^T[j, i]) so softmax reduces
    over the partition dim via the ctx matmul; V tiles carry a ones column so
    the softmax denominators fall out of the ctx matmul for free; two heads
    (D=64) packed per PE pass at partition bases 0/64.
  - fc2: k-outer loop with balanced 343/343/342-column chunks so each bf16
    weight load serves ~3 matmuls.
"""

import numpy as np
import ml_dtypes

B, S, E, H, D, II = 32, 257, 1024, 16, 64, 4096
N_CORES = 8
B_LOC = B // N_CORES          # 4
NT = B_LOC * S                # 1028
NTP = 1040                    # padded NT (chunk stride, 16B aligned)
KC = E // 128                 # 8
MC_E = E // 128               # 8
MC_I = II // 128              # 32
EPS = 1e-5
F8MAX = 240.0

# balanced column chunks of NT for batch-agnostic matmuls.  343 is the DR
# sweet spot: measured issue spacing 148ns (2 elem/cycle streaming, warm
# LDWEIGHTS hidden); N=512 DR measured 259ns (~1.65 elem/cycle) - avoid.
CH3 = [(0, 343), (343, 343), (686, 342)]
# V-projection moving chunks (output-feature dim, PSUM-bank aligned)
VCH = [(0, 256), (256, 256), (512, 256), (768, 256)]
# j-chunks of one batch element's 257 keys
JC = [(0, 128), (128, 128), (256, 1)]

TRACE = False
LAST_EXEC_NS = None

_cache = {}


def _build(with_mask: bool, with_vbias: bool, with_qkbias: bool,
           sv_inv: float, so_inv: float, sqk_inv: float, sf1_inv: float):
    import concourse.tile as tile
    from concourse import bacc, mybir
    from contextlib import ExitStack

    F32 = mybir.dt.float32
    BF16 = mybir.dt.bfloat16
    FP8 = mybir.dt.float8e4
    AF = mybir.ActivationFunctionType
    ALU = mybir.AluOpType
    DR = mybir.MatmulPerfMode.DoubleRow

    nc = bacc.Bacc("TRN2", target_bir_lowering=False, debug=False,
                   enable_asserts=False, num_devices=N_CORES)

    xT_d = nc.dram_tensor("xT", [E, NT], F32, kind="ExternalInput")
    xTb_d = nc.dram_tensor("xTb", [E, NT], BF16, kind="ExternalInput")
    qw_d = nc.dram_tensor("qw", [MC_E, 128, KC, 128], FP8, kind="ExternalInput")
    kw_d = nc.dram_tensor("kw", [MC_E, 128, KC, 128], FP8, kind="ExternalInput")
    vw_d = nc.dram_tensor("vw", [KC // 2, 128, 2, E], FP8, kind="ExternalInput")
    ow_d = nc.dram_tensor("ow", [MC_E, 128, KC, 128], FP8, kind="ExternalInput")
    f1w_d = nc.dram_tensor("f1w", [MC_I, 128, KC, 128], FP8, kind="ExternalInput")
    f2w_d = nc.dram_tensor("f2w", [MC_E, 128, MC_I, 128], BF16, kind="ExternalInput")
    qb_d = nc.dram_tensor("qb", [128, MC_E], F32, kind="ExternalInput")
    kb_d = nc.dram_tensor("kb", [128, MC_E], F32, kind="ExternalInput")
    vb_d = nc.dram_tensor("vb", [1, E], F32, kind="ExternalInput")
    f1b_d = nc.dram_tensor("f1b", [128, MC_I], F32, kind="ExternalInput")
    f2b_d = nc.dram_tensor("f2b", [128, MC_E], F32, kind="ExternalInput")
    mskT_d = None
    if with_mask:
        mskT_d = nc.dram_tensor("mskT", [B_LOC, S, S], F32, kind="ExternalInput")
    outT_d = nc.dram_tensor("outT", [E, NT], F32, kind="ExternalOutput")

    with tile.TileContext(nc) as tc, ExitStack() as top:
        consts = top.enter_context(tc.tile_pool(name="consts", bufs=1))

        ones_col = consts.tile([128, 1], BF16)
        nc.vector.memset(ones_col[:], 1.0)
        ones_row = consts.tile([1, 128], BF16)
        nc.vector.memset(ones_row[:], 1.0)
        eps_t = consts.tile([1, 1], F32)
        nc.vector.memset(eps_t[:], EPS)
        qb_sb = consts.tile([128, MC_E], F32)
        nc.sync.dma_start(out=qb_sb[:], in_=qb_d[:])
        kb_sb = consts.tile([128, MC_E], F32)
        nc.sync.dma_start(out=kb_sb[:], in_=kb_d[:])
        f2b_sb = consts.tile([128, MC_E], F32)
        nc.sync.dma_start(out=f2b_sb[:], in_=f2b_d[:])
        f1b_sb = consts.tile([128, MC_I], F32)
        nc.sync.dma_start(out=f1b_sb[:], in_=f1b_d[:])
        vb_sb = consts.tile([128, E], F32)
        nc.sync.dma_start(out=vb_sb[:], in_=vb_d[0:1, :].to_broadcast((128, E)))

        def emit_ln(ph, src_ap, srcbf_ap, x_out, sfx, sq_pre=None,
                    split_apply=False):
            """Per-batch column LayerNorm over the feature (partition) dim.
            src_ap(k, b) -> [128, S] AP (f32 or bf16); srcbf_ap(k, b) ->
            [128, S] bf16 AP.  Writes (x - mu) * rstd as fp8 into
            x_out[:, k, b*S:(b+1)*S] (LN scale/bias folded into downstream
            weights host-side).  Sum and sum-of-squares matmuls run in PE
            column groups 0 / 1 concurrently.  sq_pre: optional dict of
            pre-computed square tiles keyed (k, b).  split_apply: run half
            the normalization ops on GpSimd (via SBUF copies of the
            broadcast rows) so fc1 can start sooner."""
            lntmp = ph.enter_context(tc.tile_pool(name=f"lntmp{sfx}", bufs=3))
            sqp = ph.enter_context(tc.tile_pool(name=f"sqp{sfx}", bufs=6))
            rows = ph.enter_context(tc.tile_pool(name=f"rows{sfx}", bufs=8))
            if split_apply:
                bcs = ph.enter_context(tc.tile_pool(name=f"bcs{sfx}", bufs=4))
            pstat = ph.enter_context(
                tc.tile_pool(name=f"pstat{sfx}", bufs=2, space="PSUM"))
            pbc = ph.enter_context(
                tc.tile_pool(name=f"pbc{sfx}", bufs=2, space="PSUM"))
            sq_engines = [nc.gpsimd.tensor_mul, nc.vector.tensor_mul]
            for b in range(B_LOC):
                ps = pstat.tile([128, 512], F32, name="ps_stat", tag="stat")
                sqs = []
                for k in range(KC):
                    xb = srcbf_ap(k, b)
                    # the sum chain runs on PE while squares compute
                    nc.tensor.matmul(ps[0:1, 0:S], ones_col[:], xb,
                                     start=(k == 0), stop=(k == KC - 1),
                                     tile_position=(0, 0))
                    if sq_pre is not None:
                        sqs.append(sq_pre[(k, b)])
                    else:
                        sq = sqp.tile([128, S], BF16, name="sq", tag="sq")
                        sq_engines[k % 2](out=sq[:], in0=xb, in1=xb)
                        sqs.append(sq)
                for k in range(KC):
                    nc.tensor.matmul(ps[32:33, 0:S], ones_col[:], sqs[k][:],
                                     start=(k == 0), stop=(k == KC - 1),
                                     tile_position=(0, 32))
                musq = rows.tile([1, S], F32, name="musq", tag="row")
                nc.scalar.activation(out=musq[0:1, :], in_=ps[0:1, 0:S],
                                     func=AF.Square, scale=-1.0 / E)
                muneg_b = rows.tile([1, S], BF16, name="muneg_b", tag="row")
                nc.scalar.mul(out=muneg_b[0:1, :], in_=ps[0:1, 0:S],
                              mul=-1.0 / E)
                var = rows.tile([1, S], F32, name="var", tag="row")
                nc.vector.scalar_tensor_tensor(
                    out=var[0:1, :], in0=ps[32:33, 0:S], scalar=1.0 / E,
                    in1=musq[0:1, :], op0=ALU.mult, op1=ALU.subtract)
                sd = rows.tile([1, S], F32, name="sd", tag="row")
                nc.scalar.activation(out=sd[0:1, :], in_=var[0:1, :],
                                     func=AF.Sqrt, bias=eps_t[0:1, 0:1])
                rstd = rows.tile([1, S], F32, name="rstd", tag="row")
                nc.vector.reciprocal_approx_fast(out=rstd[0:1, :],
                                                 in_=sd[0:1, :])
                rstd_b = rows.tile([1, S], BF16, name="rstd_b", tag="row")
                nc.vector.tensor_copy(out=rstd_b[0:1, :], in_=rstd[0:1, :])
                psA = pbc.tile([128, 512], F32, name="psA", tag="bc")
                psB = pbc.tile([128, 512], F32, name="psB", tag="bc")
                nc.tensor.matmul(psA[:, 0:S], ones_row[0:1, :],
                                 rstd_b[0:1, :], start=True, stop=True)
                nc.tensor.matmul(psB[:, 0:S], ones_row[0:1, :],
                                 muneg_b[0:1, :], start=True, stop=True)
                if split_apply and b >= 2:
                    # GpSimd cannot read PSUM: stage the broadcast rows in
                    # SBUF, then normalize on GpSimd to keep DVE free
                    sbA = bcs.tile([128, S], F32, name="sbA", tag="bcs")
                    nc.scalar.copy(out=sbA[:], in_=psA[:, 0:S])
                    sbB = bcs.tile([128, S], F32, name="sbB", tag="bcs")
                    nc.scalar.copy(out=sbB[:], in_=psB[:, 0:S])
                    for k in range(KC):
                        tmp = lntmp.tile([128, S], F32, name="tmp", tag="ap")
                        nc.gpsimd.tensor_add(out=tmp[:], in0=src_ap(k, b),
                                             in1=sbB[:])
                        nc.gpsimd.tensor_mul(
                            out=x_out[:, k, b * S:(b + 1) * S],
                            in0=tmp[:], in1=sbA[:])
                else:
                    for k in range(KC):
                        tmp = lntmp.tile([128, S], F32, name="tmp", tag="ap")
                        nc.vector.tensor_add(out=tmp[:], in0=src_ap(k, b),
                                             in1=psB[:, 0:S])
                        nc.vector.tensor_mul(
                            out=x_out[:, k, b * S:(b + 1) * S],
                            in0=tmp[:], in1=psA[:, 0:S])

        # fp8 chunk-major activations: [128, KC, NTP]
        x1_p = top.enter_context(tc.tile_pool(name="x1", bufs=1))
        x1 = x1_p.tile([128, KC, NTP], FP8, name="x1", tag="x1")
        x2_p = top.enter_context(tc.tile_pool(name="x2", bufs=1))
        x2 = x2_p.tile([128, KC, NTP], FP8, name="x2", tag="x2")

        # residual (x + o_b)^T f32: emitted on the Scalar engine's DMA
        # queue so weight DMAs (Sync queue) are not stuck behind these
        # 4.2MB; loads overlap LN1/V/attention.
        xt_p = top.enter_context(tc.tile_pool(name="xt", bufs=KC))
        xt = []
        for k in range(KC):
            t = xt_p.tile([128, NT], F32, name="xt", tag="xt")
            nc.scalar.dma_start(
                out=t[:], in_=xT_d[k * 128:(k + 1) * 128, :])
            xt.append(t)

        # long-lived right-side pools, opened in close order (LIFO):
        ht_p = top.enter_context(
            tc.tile_pool(name="ht", bufs=KC, side="right"))
        # closed explicitly after LN2 (holds bf16 copies + squares)
        ln2prep_ph = ExitStack()
        htb_p = ln2prep_ph.enter_context(
            tc.tile_pool(name="htb", bufs=KC, side="right"))
        sq2_p = ln2prep_ph.enter_context(
            tc.tile_pool(name="sq2", bufs=KC * B_LOC, side="right"))

        with tc.tile_pool(name="ctx2", bufs=MC_E // 2) as ctx_p:
            ctx2 = [ctx_p.tile([128, 2, NTP], FP8, tag="ctx2", name="ctx2")
                    for _ in range(MC_E // 2)]

            # ============= LN1 / V / QK+attention ====================
            with tc.tile_pool(name="vpool", bufs=2 * B_LOC + 1) as v_p:
                with ExitStack() as ln1_ph:
                    xtb_p = ln1_ph.enter_context(
                        tc.tile_pool(name="xtb", bufs=KC * B_LOC))
                    xtb = {}
                    for b in range(B_LOC):
                        for k in range(KC):
                            tb = xtb_p.tile([128, S], BF16, name="xtb",
                                            tag="xtb")
                            nc.sync.dma_start(
                                out=tb[:],
                                in_=xTb_d[k * 128:(k + 1) * 128,
                                          b * S:(b + 1) * S])
                            xtb[(k, b)] = tb
                    emit_ln(ln1_ph,
                            lambda k, b: xtb[(k, b)][:],
                            lambda k, b: xtb[(k, b)][:],
                            x1, "1")

                    # ============= V projection (fp8 DoubleRow) ==========
                    v_tiles = {}
                    with ExitStack() as ph:
                        vw_p = ph.enter_context(
                            tc.tile_pool(name="vw", bufs=KC // 2))
                        ppv = ph.enter_context(
                            tc.tile_pool(name="ppv", bufs=2, space="PSUM"))
                        vw_sb = []
                        for kk in range(KC // 2):
                            vwk = vw_p.tile([128, 2, E], FP8, name="vwk",
                                            tag="vwk")
                            nc.sync.dma_start(out=vwk[:], in_=vw_d[kk, :, :, :])
                            vw_sb.append(vwk)
                        for b in range(B_LOC):
                            for jc, (j0, jcs) in enumerate(JC[:2]):
                                ps = ppv.tile([128, 2, 512], F32,
                                              name="vps", tag="vps")
                                # n-outer: a start=True clears has_written
                                # for the whole bank, so only one
                                # accumulation chain may be live per bank
                                for n, (n0, nn) in enumerate(VCH):
                                    for kk in range(KC // 2):
                                        nc.tensor.matmul(
                                            ps[0:jcs, n0 // 512,
                                               n0 % 512:n0 % 512 + nn],
                                            x1[:, 2 * kk:2 * kk + 2,
                                               b * S + j0:b * S + j0 + jcs],
                                            vw_sb[kk][:, :, n0:n0 + nn],
                                            start=(kk == 0),
                                            stop=(kk == KC // 2 - 1),
                                            perf_mode=DR)
                                # [tok, H, 128]: cols 0:64 ones, cols
                                # 64:128 V -> ctx matmul replicates the
                                # softmax sums across partitions 0:64.
                                vt = v_p.tile([128, H, 128], BF16,
                                              name="vt", tag="vt")
                                if with_vbias:
                                    nc.vector.scalar_tensor_tensor(
                                        out=vt[0:jcs, :, 64:128],
                                        in0=ps[0:jcs, :, :], scalar=sv_inv,
                                        in1=vb_sb[0:jcs, :],
                                        op0=ALU.mult, op1=ALU.add)
                                else:
                                    nc.scalar.mul(
                                        out=vt[0:jcs, :, 64:128],
                                        in_=ps[0:jcs, :, :], mul=sv_inv)
                                nc.gpsimd.memset(vt[:, :, 0:64], 1.0)
                                v_tiles[(b, jc)] = vt
                        # the 4 batches' tail token (j=256): pack the
                        # M=1 matmuls into column groups 0/32/64/96 so
                        # they run concurrently on the PE array.
                        ps = ppv.tile([128, 2, 512], F32,
                                      name="vps_t", tag="vps")
                        for b in range(B_LOC):
                            for n in range(2):
                                for k in range(KC):
                                    nc.tensor.matmul(
                                        ps[32 * b:32 * b + 1, n, :],
                                        x1[:, k, b * S + 256:b * S + 257],
                                        vw_sb[k // 2][:, k % 2,
                                                      n * 512:(n + 1) * 512],
                                        start=(k == 0), stop=(k == KC - 1),
                                        tile_position=(0, 32 * b))
                        vt_t = v_p.tile([128, H, 128], BF16,
                                        name="vt_t", tag="vt")
                        nc.gpsimd.memset(vt_t[:, :, 0:64], 1.0)
                        for b in range(B_LOC):
                            if with_vbias:
                                nc.vector.scalar_tensor_tensor(
                                    out=vt_t[32 * b:32 * b + 1, :, 64:128],
                                    in0=ps[32 * b:32 * b + 1, :, :],
                                    scalar=sv_inv, in1=vb_sb[0:1, :],
                                    op0=ALU.mult, op1=ALU.add)
                            else:
                                nc.scalar.mul(
                                    out=vt_t[32 * b:32 * b + 1, :, 64:128],
                                    in_=ps[32 * b:32 * b + 1, :, :],
                                    mul=sv_inv)
                            v_tiles[(b, 2)] = vt_t

                # ========= Q/K + attention (per head-pair chunk) =====
                with ExitStack() as ph:
                    qt_p = ph.enter_context(tc.tile_pool(name="qt", bufs=2))
                    kt_p = ph.enter_context(tc.tile_pool(name="kt", bufs=2))
                    wqk_p = ph.enter_context(
                        tc.tile_pool(name="wqk", bufs=4))
                    e_p = ph.enter_context(tc.tile_pool(name="ep", bufs=9))
                    rs_p = ph.enter_context(tc.tile_pool(name="rsp", bufs=4))
                    if with_mask:
                        msk_p = ph.enter_context(
                            tc.tile_pool(name="mskp", bufs=3 * B_LOC))
                    pp2 = ph.enter_context(
                        tc.tile_pool(name="pp2", bufs=1, space="PSUM"))
                    psp = ph.enter_context(
                        tc.tile_pool(name="psp", bufs=3, space="PSUM"))
                    if with_mask:
                        msk = {}
                        for b in range(B_LOC):
                            for jc, (j0, jcs) in enumerate(JC):
                                mt = msk_p.tile([128, S], F32, name="mt",
                                                tag="mt")
                                nc.sync.dma_start(
                                    out=mt[0:jcs, :],
                                    in_=mskT_d[b, j0:j0 + jcs, :])
                                msk[(b, jc)] = mt

                    for ec in range(MC_E):
                        # ps_t serves two roles this iteration: first its
                        # banks hold the Q/K projections' third column
                        # chunk, then rows 32b get the tail-key scores.
                        ps_t = psp.tile([128, 2, 512], F32,
                                        name="ps_t", tag="sp")
                        nc.vector.memset(ps_t[:], 0.0)
                        qkt = []
                        for ip, (w_d, b_sb, opool) in enumerate((
                                (qw_d, qb_sb, qt_p),
                                (kw_d, kb_sb, kt_p))):
                            wt = wqk_p.tile([128, KC, 128], FP8,
                                            name="wqk", tag="wqk")
                            nc.sync.dma_start(out=wt[:],
                                              in_=w_d[ec, :, :, :])
                            ot = opool.tile([128, NT], BF16,
                                            name="qk", tag="qk")
                            ps = pp2.tile([128, 2, 512], F32,
                                          name="pqk", tag="pqk")
                            for c, (c0, cn) in enumerate(CH3):
                                dst = (ps[:, c, 0:cn] if c < 2
                                       else ps_t[:, ip, 0:cn])
                                for kk in range(KC // 2):
                                    nc.tensor.matmul(
                                        dst,
                                        wt[:, 2 * kk:2 * kk + 2, :],
                                        x1[:, 2 * kk:2 * kk + 2,
                                           c0:c0 + cn],
                                        start=(kk == 0),
                                        stop=(kk == KC // 2 - 1),
                                        perf_mode=DR)
                            # drains split across ACT and DVE so the
                            # single pp2 buffer frees quickly
                            drains = [(ps[:, 0, 0:CH3[0][1]],) + CH3[0],
                                      (ps[:, 1, 0:CH3[1][1]],) + CH3[1],
                                      (ps_t[:, ip, 0:CH3[2][1]],) + CH3[2]]
                            for c, (src, c0, cn) in enumerate(drains):
                                if with_qkbias:
                                    nc.vector.tensor_scalar_add(
                                        out=ot[:, c0:c0 + cn], in0=src,
                                        scalar1=b_sb[:, ec:ec + 1])
                                elif (c + ip) % 2 == 0:
                                    nc.scalar.copy(out=ot[:, c0:c0 + cn],
                                                   in_=src)
                                else:
                                    nc.vector.tensor_copy(
                                        out=ot[:, c0:c0 + cn], in_=src)
                            qkt.append(ot)
                        qte, kte = qkt

                        # tail key (j=256) for all 4 batches: packed
                        # into array col groups 32b / row groups 64*hi,
                        # one shared exp over all rows.
                        for b in range(B_LOC):
                            for hi in range(2):
                                p0 = hi * 64
                                nc.tensor.matmul(
                                    ps_t[32 * b:32 * b + 1, hi, 0:S],
                                    kte[p0:p0 + 64,
                                        b * S + 256: b * S + 257],
                                    qte[p0:p0 + 64, b * S:(b + 1) * S],
                                    start=True, stop=True,
                                    tile_position=(p0, 32 * b))
                        if with_mask:
                            for b in range(B_LOC):
                                for hi in range(2):
                                    nc.vector.tensor_add(
                                        out=ps_t[32 * b:32 * b + 1, hi, 0:S],
                                        in0=ps_t[32 * b:32 * b + 1, hi, 0:S],
                                        in1=msk[(b, 2)][0:1, :])
                        et_t = e_p.tile([128, 2, S], BF16,
                                        name="et_t", tag="et")
                        nc.scalar.activation(out=et_t[:, :, :],
                                             in_=ps_t[:, :, 0:S],
                                             func=AF.Exp, scale=sqk_inv)

                        for b in range(B_LOC):
                            ets = []
                            for jc, (j0, jcs) in enumerate(JC[:2]):
                                sp = psp.tile([128, 2, 512], F32,
                                              name="sp", tag="sp")
                                for hi in range(2):
                                    p0 = hi * 64
                                    nc.tensor.matmul(
                                        sp[0:jcs, hi, 0:S],
                                        kte[p0:p0 + 64,
                                            b * S + j0: b * S + j0 + jcs],
                                        qte[p0:p0 + 64,
                                            b * S:(b + 1) * S],
                                        start=True, stop=True)
                                if with_mask:
                                    for hi in range(2):
                                        nc.vector.tensor_add(
                                            out=sp[0:jcs, hi, 0:S],
                                            in0=sp[0:jcs, hi, 0:S],
                                            in1=msk[(b, jc)][0:jcs, :])
                                et = e_p.tile([128, 2, S], BF16,
                                              name="et", tag="et")
                                nc.scalar.activation(
                                    out=et[0:jcs, :, :],
                                    in_=sp[0:jcs, :, 0:S], func=AF.Exp,
                                    scale=sqk_inv)
                                ets.append(et)
                            cp = psp.tile([128, 2, 512], F32,
                                          name="cp", tag="sp")
                            for hi in range(2):
                                h = 2 * ec + hi
                                for jc, (j0, jcs) in enumerate(JC[:2]):
                                    nc.tensor.matmul(
                                        cp[0:128, hi, 0:S],
                                        v_tiles[(b, jc)][0:jcs, h, :],
                                        ets[jc][0:jcs, hi, :],
                                        start=(jc == 0), stop=False)
                                nc.tensor.matmul(
                                    cp[0:128, hi, 0:S],
                                    v_tiles[(b, 2)][32 * b:32 * b + 1,
                                                    h, :],
                                    et_t[32 * b:32 * b + 1, hi, :],
                                    start=False, stop=True,
                                    tile_position=(32 * b, 0))
                            rst = rs_p.tile([64, 2, S], F32,
                                            name="rst", tag="rst")
                            nc.vector.reciprocal_approx_fast(
                                out=rst[0:64, :, :],
                                in_=cp[0:64, :, 0:S])
                            for hi in range(2):
                                nc.vector.tensor_mul(
                                    out=ctx2[ec // 2][hi * 64:hi * 64 + 64,
                                                      ec % 2,
                                                      b * S:(b + 1) * S],
                                    in0=cp[64:128, hi, 0:S],
                                    in1=rst[0:64, hi, :])

            # ===== out projection (fp8 DoubleRow) + residual =====
            # LN2 prep (bf16 copy + squares) is interleaved per m-tile so
            # the LN2 stats can start the moment the projection finishes.
            ht, htb, sq2 = [], [], {}
            with ExitStack() as ph:
                wo_p = ph.enter_context(tc.tile_pool(name="wo", bufs=3))
                ppo = ph.enter_context(
                    tc.tile_pool(name="ppo", bufs=2, space="PSUM"))
                for m in range(MC_E):
                    wt = wo_p.tile([128, KC, 128], FP8, name="wo",
                                   tag="wo")
                    nc.sync.dma_start(out=wt[:], in_=ow_d[m, :, :, :])
                    ps = ppo.tile([128, 3, 512], F32, name="po",
                                  tag="po")
                    for c, (c0, cn) in enumerate(CH3):
                        for kk in range(KC // 2):
                            nc.tensor.matmul(
                                ps[:, c, 0:cn],
                                wt[:, 2 * kk:2 * kk + 2, :],
                                ctx2[kk][:, :, c0:c0 + cn],
                                start=(kk == 0), stop=(kk == KC // 2 - 1),
                                perf_mode=DR)
                    o = ht_p.tile([128, NT], F32, name="ht", tag="ht")
                    for c, (c0, cn) in enumerate(CH3):
                        nc.vector.scalar_tensor_tensor(
                            out=o[:, c0:c0 + cn],
                            in0=ps[:, c, 0:cn],
                            scalar=so_inv,
                            in1=xt[m][:, c0:c0 + cn],
                            op0=ALU.mult, op1=ALU.add)
                    ht.append(o)
                    hb = htb_p.tile([128, NT], BF16, name="htb", tag="htb")
                    nc.vector.tensor_copy(out=hb[:], in_=o[:])
                    htb.append(hb)
                    for b in range(B_LOC):
                        sq = sq2_p.tile([128, S], BF16, name="sq2",
                                        tag="sq2")
                        if b % 2 == 0:
                            nc.scalar.activation(
                                out=sq[:], in_=hb[:, b * S:(b + 1) * S],
                                func=AF.Square)
                        else:
                            nc.gpsimd.tensor_mul(
                                out=sq[:], in0=hb[:, b * S:(b + 1) * S],
                                in1=hb[:, b * S:(b + 1) * S])
                        sq2[(m, b)] = sq
        # ctx2 closed

        # ================= LN2 + MLP =====================================
        with ExitStack() as ln2_ph:
            emit_ln(ln2_ph,
                    lambda k, b: ht[k][:, b * S:(b + 1) * S],
                    lambda k, b: htb[k][:, b * S:(b + 1) * S],
                    x2, "2", sq_pre=sq2, split_apply=True)
        ln2prep_ph.close()
        # f1o reuses the SBUF freed by htb/sq2
        f1o_p = top.enter_context(
            tc.tile_pool(name="f1o", bufs=MC_I, side="right"))

        # ============= fc1 (fp8 DoubleRow) ==============================
        f1o = []
        with ExitStack() as ph:
            # fc2 weight pool opens early so the first fc2 weights prefetch
            # during fc1
            wf2_p = ph.enter_context(tc.tile_pool(name="wf2", bufs=2))
            f2w_t = {}
            with ExitStack() as ph1:
                wf1_p = ph1.enter_context(tc.tile_pool(name="wf1", bufs=4))
                ppf1 = ph1.enter_context(
                    tc.tile_pool(name="ppf1", bufs=2, space="PSUM"))
                for m in range(MC_I):
                    wt = wf1_p.tile([128, KC, 128], FP8, name="wf1",
                                    tag="wf1")
                    nc.sync.dma_start(out=wt[:], in_=f1w_d[m, :, :, :])
                    o = f1o_p.tile([128, NT], BF16, name="f1o", tag="f1o")
                    ps = ppf1.tile([128, 3, 512], F32, name="pf1", tag="pf1")
                    for c, (c0, cn) in enumerate(CH3):
                        for kk in range(KC // 2):
                            nc.tensor.matmul(
                                ps[:, c, 0:cn],
                                wt[:, 2 * kk:2 * kk + 2, :],
                                x2[:, 2 * kk:2 * kk + 2, c0:c0 + cn],
                                start=(kk == 0), stop=(kk == KC // 2 - 1),
                                perf_mode=DR)
                    for c, (c0, cn) in enumerate(CH3):
                        nc.scalar.activation(
                            out=o[:, c0:c0 + cn],
                            in_=ps[:, c, 0:cn],
                            func=AF.Gelu_apprx_tanh, scale=sf1_inv,
                            bias=f1b_sb[:, m:m + 1])
                    f1o.append(o)
                    if m == MC_I // 2:
                        # kick off fc2 weight DMAs mid-fc1 so the fc2 loop
                        # starts hot
                        for mm in range(2):
                            wt2 = wf2_p.tile([128, MC_I, 128], BF16,
                                             name="wf2", tag="wf2")
                            nc.sync.dma_start(out=wt2[:],
                                              in_=f2w_d[mm, :, :, :])
                            f2w_t[mm] = wt2

            # ============= fc2 (bf16, k-outer, 343-col chunks) ==========
            with ExitStack() as ph2:
                ppf2 = ph2.enter_context(
                    tc.tile_pool(name="ppf2", bufs=2, space="PSUM"))
                out_p = ph2.enter_context(tc.tile_pool(name="outp", bufs=3))
                for m in range(MC_E):
                    if m not in f2w_t:
                        wt2 = wf2_p.tile([128, MC_I, 128], BF16,
                                         name="wf2", tag="wf2")
                        nc.sync.dma_start(out=wt2[:], in_=f2w_d[m, :, :, :])
                        f2w_t[m] = wt2
                    wt = f2w_t.pop(m)
                    ps = ppf2.tile([128, 3, 512], F32, name="pf2", tag="pf2")
                    for k in range(MC_I):
                        for c, (c0, cn) in enumerate(CH3):
                            nc.tensor.matmul(
                                ps[:, c, 0:cn], wt[:, k, :],
                                f1o[k][:, c0:c0 + cn],
                                start=(k == 0), stop=(k == MC_I - 1))
                    o = out_p.tile([128, NT], F32, name="oo", tag="oo")
                    for c, (c0, cn) in enumerate(CH3):
                        nc.vector.scalar_tensor_tensor(
                            out=o[:, c0:c0 + cn], in0=ps[:, c, 0:cn],
                            scalar=f2b_sb[:, m:m + 1],
                            in1=ht[m][:, c0:c0 + cn],
                            op0=ALU.add, op1=ALU.add)
                    nc.sync.dma_start(out=outT_d[m * 128:(m + 1) * 128, :],
                                      in_=o[:])

    nc.compile()
    return nc


def _pow2_scale(W):
    """Largest power-of-two s with max|W|*s <= F8MAX."""
    m = float(np.abs(W).max())
    if m == 0.0:
        return 1.0
    return 2.0 ** np.floor(np.log2(F8MAX / m))


def _pack_lhsT8(W, s):
    """W [M, K] (out, in) -> [M/128, 128, K/128, 128] fp8 with
    [m, p, k, j] = W[m*128+j, k*128+p] * s (lhsT tiles, partition = K)."""
    W = np.asarray(W, np.float32) * s
    M, K = W.shape
    A = W.reshape(M // 128, 128, K // 128, 128)
    A = np.ascontiguousarray(A.transpose(0, 3, 2, 1))
    return np.clip(A, -F8MAX, F8MAX).astype(ml_dtypes.float8_e4m3)


def _pack_lhsT(W):
    """W [M, K] -> [M/128, 128, K/128, 128] bf16 lhsT tiles."""
    W = np.asarray(W, np.float32)
    M, K = W.shape
    A = W.reshape(M // 128, 128, K // 128, 128)
    return np.ascontiguousarray(A.transpose(0, 3, 2, 1)).astype(ml_dtypes.bfloat16)


def _pack_pbias(b):
    """b [M] -> [128, M/128] f32 per-partition bias columns."""
    return np.ascontiguousarray(np.asarray(b, np.float32).reshape(-1, 128).T)


def kernel(hidden_states, attention_mask, causal_attention_mask,
           ln1_w, ln1_b, q_w, q_b, k_w, k_b, v_w, v_b, o_w, o_b,
           ln2_w, ln2_b, fc1_w, fc1_b, fc2_w, fc2_b):
    global LAST_EXEC_NS
    from concourse.bass_utils import run_bass_kernel_spmd

    hs = np.asarray(hidden_states, np.float32)
    msk = (np.asarray(attention_mask, np.float32)
           + np.asarray(causal_attention_mask, np.float32))
    with_mask = bool(np.any(msk))

    ln1_w = np.asarray(ln1_w, np.float32); ln1_b = np.asarray(ln1_b, np.float32)
    ln2_w = np.asarray(ln2_w, np.float32); ln2_b = np.asarray(ln2_b, np.float32)
    q_w = np.asarray(q_w, np.float32); q_b = np.asarray(q_b, np.float32)
    k_w = np.asarray(k_w, np.float32); k_b = np.asarray(k_b, np.float32)
    v_w = np.asarray(v_w, np.float32); v_b = np.asarray(v_b, np.float32)
    o_w = np.asarray(o_w, np.float32); o_b = np.asarray(o_b, np.float32)
    fc1_w = np.asarray(fc1_w, np.float32); fc1_b = np.asarray(fc1_b, np.float32)
    fc2_w = np.asarray(fc2_w, np.float32); fc2_b = np.asarray(fc2_b, np.float32)

    scale = D ** -0.5
    # fold LN1 scale/bias into Q/K/V, and the softmax scale into Q
    qw_eff = (q_w * ln1_w[None, :]) * scale
    qb_eff = (q_b + q_w @ ln1_b) * scale
    kw_eff = k_w * ln1_w[None, :]
    kb_eff = k_b + k_w @ ln1_b
    vw_eff = v_w * ln1_w[None, :]
    vb_eff = v_b + v_w @ ln1_b
    # fold LN2 into fc1
    f1w_eff = fc1_w * ln2_w[None, :]
    f1b_eff = fc1_b + fc1_w @ ln2_b

    # fp8 power-of-two scales
    s_q = _pow2_scale(qw_eff)
    s_k = _pow2_scale(kw_eff)
    s_v = _pow2_scale(vw_eff)
    s_o = _pow2_scale(o_w)
    s_f1 = _pow2_scale(f1w_eff)

    vw8 = np.clip(vw_eff.T * s_v, -F8MAX, F8MAX).astype(
        ml_dtypes.float8_e4m3).reshape(KC // 2, 2, 128, E).transpose(0, 2, 1, 3)

    base = {
        "qw": _pack_lhsT8(qw_eff, s_q),
        "kw": _pack_lhsT8(kw_eff, s_k),
        "vw": np.ascontiguousarray(vw8),
        "ow": _pack_lhsT8(o_w, s_o),
        "f1w": _pack_lhsT8(f1w_eff, s_f1),
        "f2w": _pack_lhsT(fc2_w),
        "qb": _pack_pbias(qb_eff * s_q),
        "kb": _pack_pbias(kb_eff * s_k),
        "vb": np.ascontiguousarray(vb_eff[None, :].astype(np.float32)),
        "f1b": _pack_pbias(f1b_eff),
        "f2b": _pack_pbias(fc2_b),
    }

    with_vbias = bool(np.any(vb_eff))
    with_qkbias = bool(np.any(qb_eff)) or bool(np.any(kb_eff))
    key = (with_mask, with_vbias, with_qkbias, s_q, s_k, s_v, s_o, s_f1)
    if key not in _cache:
        _cache[key] = _build(with_mask, with_vbias, with_qkbias,
                             1.0 / s_v, 1.0 / s_o, 1.0 / (s_q * s_k),
                             1.0 / s_f1)
    nc = _cache[key]

    # o_b folded into the residual
    res = hs + o_b[None, None, :]

    in_maps = []
    for c in range(N_CORES):
        x = hs[c * B_LOC:(c + 1) * B_LOC].reshape(NT, E).T
        r = res[c * B_LOC:(c + 1) * B_LOC].reshape(NT, E).T
        m = dict(base)
        m["xT"] = np.ascontiguousarray(r)
        m["xTb"] = np.ascontiguousarray(x).astype(ml_dtypes.bfloat16)
        if with_mask:
            m["mskT"] = np.ascontiguousarray(
                msk[c * B_LOC:(c + 1) * B_LOC, 0].transpose(0, 2, 1)
                * (s_q * s_k))
        in_maps.append(m)

    res_k = run_bass_kernel_spmd(nc, in_maps, core_ids=list(range(N_CORES)),
                                 trace=TRACE)
    LAST_EXEC_NS = res_k.exec_time_ns

    outs = []
    for c in range(N_CORES):
        oT = res_k.results[c]["outT"]          # [E, NT] f32
        outs.append(np.ascontiguousarray(oT.T).reshape(B_LOC, S, E))
    return np.concatenate(outs, axis=0)


# revision 41
# speedup vs baseline: 1.2516x; 1.0406x over previous
"""Trainium2 Bass kernel for a CLIP encoder layer (B=32, S=257, E=1024, H=16, I=4096).

Strategy: data-parallel over batch across 8 NeuronCores (4 batch elements per
core), no collectives.  Per-core compute is feature-major ([E, tokens]) so
projection matmuls need no on-device transposes.

v2: Q/K/V/out-proj and fc1 run in fp8(e4m3) with DoubleRow perf mode (two
128-feature chunks contracted per PE pass).  Weights are scaled by a power of
two into the e4m3 range on the host; the de-scale is folded into cheap spots:
  - q/k: scores are computed on scaled q/k, exp() applies 1/(s_q*s_k) via the
    ACT scale input.
  - v: the PSUM->SBUF copy multiplies by 1/s_v.
  - out-proj: the residual-add STT multiplies by 1/s_o; o_b is folded into the
    residual x^T on the host.
  - fc1: gelu's ACT scale input applies 1/s_f1.
fc2 stays bf16 (fp8 there pushes rel-err past the 2e-2 gate).

Other structure:
  - LayerNorm column stats via PE ones-matmuls; sum and sum-of-squares run
    concurrently in different PE column groups (tile_position).  Squares are
    computed on GpSimd.  LN scale/bias folded into downstream weights host-side.
  - LN outputs written as fp8 into [128, KC, 1040] feature-chunk-major tiles so
    DoubleRow matmuls can take chunk-pair APs directly.
  - Attention: scores computed transposed (scores^T[j, i]) so softmax reduces
    over the partition dim via the ctx matmul; V tiles carry a ones column so
    the softmax denominators fall out of the ctx matmul for free; two heads
    (D=64) packed per PE pass at partition bases 0/64.
  - fc2: k-outer loop with balanced 343/343/342-column chunks so each bf16
    weight load serves ~3 matmuls.
"""

import numpy as np
import ml_dtypes

B, S, E, H, D, II = 32, 257, 1024, 16, 64, 4096
N_CORES = 8
B_LOC = B // N_CORES          # 4
NT = B_LOC * S                # 1028
NTP = 1040                    # padded NT (chunk stride, 16B aligned)
KC = E // 128                 # 8
MC_E = E // 128               # 8
MC_I = II // 128              # 32
EPS = 1e-5
F8MAX = 240.0

# balanced column chunks of NT for batch-agnostic matmuls.  343 is the DR
# sweet spot: measured issue spacing 148ns (2 elem/cycle streaming, warm
# LDWEIGHTS hidden); N=512 DR measured 259ns (~1.65 elem/cycle) - avoid.
CH3 = [(0, 343), (343, 343), (686, 342)]
# V-projection moving chunks (output-feature dim, PSUM-bank aligned)
VCH = [(0, 256), (256, 256), (512, 256), (768, 256)]
# j-chunks of one batch element's 257 keys
JC = [(0, 128), (128, 128), (256, 1)]

TRACE = False
LAST_EXEC_NS = None

_cache = {}


def _build(with_mask: bool, with_vbias: bool, with_qkbias: bool,
           sv_inv: float, so_inv: float, sqk_inv: float, sf1_inv: float):
    import concourse.tile as tile
    from concourse import bacc, mybir
    from contextlib import ExitStack

    F32 = mybir.dt.float32
    BF16 = mybir.dt.bfloat16
    FP8 = mybir.dt.float8e4
    AF = mybir.ActivationFunctionType
    ALU = mybir.AluOpType
    DR = mybir.MatmulPerfMode.DoubleRow

    nc = bacc.Bacc("TRN2", target_bir_lowering=False, debug=False,
                   enable_asserts=False, num_devices=N_CORES)

    xT_d = nc.dram_tensor("xT", [E, NT], F32, kind="ExternalInput")
    xTb_d = nc.dram_tensor("xTb", [E, NT], BF16, kind="ExternalInput")
    qw_d = nc.dram_tensor("qw", [MC_E, 128, KC, 128], FP8, kind="ExternalInput")
    kw_d = nc.dram_tensor("kw", [MC_E, 128, KC, 128], FP8, kind="ExternalInput")
    vw_d = nc.dram_tensor("vw", [KC // 2, 128, 2, E], FP8, kind="ExternalInput")
    ow_d = nc.dram_tensor("ow", [MC_E, 128, KC, 128], FP8, kind="ExternalInput")
    f1w_d = nc.dram_tensor("f1w", [MC_I, 128, KC, 128], FP8, kind="ExternalInput")
    f2w_d = nc.dram_tensor("f2w", [MC_E, 128, MC_I, 128], BF16, kind="ExternalInput")
    qb_d = nc.dram_tensor("qb", [128, MC_E], F32, kind="ExternalInput")
    kb_d = nc.dram_tensor("kb", [128, MC_E], F32, kind="ExternalInput")
    vb_d = nc.dram_tensor("vb", [1, E], F32, kind="ExternalInput")
    f1b_d = nc.dram_tensor("f1b", [128, MC_I], F32, kind="ExternalInput")
    f2b_d = nc.dram_tensor("f2b", [128, MC_E], F32, kind="ExternalInput")
    mskT_d = None
    if with_mask:
        mskT_d = nc.dram_tensor("mskT", [B_LOC, S, S], F32, kind="ExternalInput")
    outT_d = nc.dram_tensor("outT", [E, NT], F32, kind="ExternalOutput")

    with tile.TileContext(nc) as tc, ExitStack() as top:
        consts = top.enter_context(tc.tile_pool(name="consts", bufs=1))

        ones_col = consts.tile([128, 1], BF16)
        nc.vector.memset(ones_col[:], 1.0)
        ones_row = consts.tile([1, 128], BF16)
        nc.vector.memset(ones_row[:], 1.0)
        eps_t = consts.tile([1, 1], F32)
        nc.vector.memset(eps_t[:], EPS)
        qb_sb = consts.tile([128, MC_E], F32)
        nc.sync.dma_start(out=qb_sb[:], in_=qb_d[:])
        kb_sb = consts.tile([128, MC_E], F32)
        nc.sync.dma_start(out=kb_sb[:], in_=kb_d[:])
        f2b_sb = consts.tile([128, MC_E], F32)
        nc.sync.dma_start(out=f2b_sb[:], in_=f2b_d[:])
        f1b_sb = consts.tile([128, MC_I], F32)
        nc.sync.dma_start(out=f1b_sb[:], in_=f1b_d[:])
        vb_sb = consts.tile([128, E], F32)
        nc.sync.dma_start(out=vb_sb[:], in_=vb_d[0:1, :].to_broadcast((128, E)))

        def emit_ln(ph, src_ap, srcbf_ap, x_out, sfx, sq_pre=None,
                    split_apply=False):
            """Per-batch column LayerNorm over the feature (partition) dim.
            src_ap(k, b) -> [128, S] AP (f32 or bf16); srcbf_ap(k, b) ->
            [128, S] bf16 AP.  Writes (x - mu) * rstd as fp8 into
            x_out[:, k, b*S:(b+1)*S] (LN scale/bias folded into downstream
            weights host-side).  Sum and sum-of-squares matmuls run in PE
            column groups 0 / 1 concurrently.  sq_pre: optional dict of
            pre-computed square tiles keyed (k, b).  split_apply: run half
            the normalization ops on GpSimd (via SBUF copies of the
            broadcast rows) so fc1 can start sooner."""
            lntmp = ph.enter_context(tc.tile_pool(name=f"lntmp{sfx}", bufs=4))
            sqp = ph.enter_context(tc.tile_pool(name=f"sqp{sfx}", bufs=6))
            rows = ph.enter_context(tc.tile_pool(name=f"rows{sfx}", bufs=8))
            bcs = ph.enter_context(tc.tile_pool(name=f"bcs{sfx}", bufs=4))
            BCD = BF16
            pstat = ph.enter_context(
                tc.tile_pool(name=f"pstat{sfx}", bufs=2, space="PSUM"))
            pbc = ph.enter_context(
                tc.tile_pool(name=f"pbc{sfx}", bufs=2, space="PSUM"))
            sq_engines = [nc.gpsimd.tensor_mul, nc.vector.tensor_mul]
            for b in range(B_LOC):
                ps = pstat.tile([128, 512], F32, name="ps_stat", tag="stat")
                sqs = []
                for k in range(KC):
                    xb = srcbf_ap(k, b)
                    # the sum chain runs on PE while squares compute
                    nc.tensor.matmul(ps[0:1, 0:S], ones_col[:], xb,
                                     start=(k == 0), stop=(k == KC - 1),
                                     tile_position=(0, 0))
                    if sq_pre is not None:
                        sqs.append(sq_pre[(k, b)])
                    else:
                        sq = sqp.tile([128, S], BF16, name="sq", tag="sq")
                        sq_engines[k % 2](out=sq[:], in0=xb, in1=xb)
                        sqs.append(sq)
                for k in range(KC):
                    nc.tensor.matmul(ps[32:33, 0:S], ones_col[:], sqs[k][:],
                                     start=(k == 0), stop=(k == KC - 1),
                                     tile_position=(0, 32))
                musq = rows.tile([1, S], F32, name="musq", tag="row")
                nc.scalar.activation(out=musq[0:1, :], in_=ps[0:1, 0:S],
                                     func=AF.Square, scale=-1.0 / E)
                muneg_b = rows.tile([1, S], BF16, name="muneg_b", tag="row")
                nc.scalar.mul(out=muneg_b[0:1, :], in_=ps[0:1, 0:S],
                              mul=-1.0 / E)
                var = rows.tile([1, S], F32, name="var", tag="row")
                nc.vector.scalar_tensor_tensor(
                    out=var[0:1, :], in0=ps[32:33, 0:S], scalar=1.0 / E,
                    in1=musq[0:1, :], op0=ALU.mult, op1=ALU.subtract)
                sd = rows.tile([1, S], F32, name="sd", tag="row")
                nc.scalar.activation(out=sd[0:1, :], in_=var[0:1, :],
                                     func=AF.Sqrt, bias=eps_t[0:1, 0:1])
                rstd = rows.tile([1, S], F32, name="rstd", tag="row")
                nc.vector.reciprocal_approx_fast(out=rstd[0:1, :],
                                                 in_=sd[0:1, :])
                rstd_b = rows.tile([1, S], BF16, name="rstd_b", tag="row")
                nc.vector.tensor_copy(out=rstd_b[0:1, :], in_=rstd[0:1, :])
                psA = pbc.tile([128, 512], F32, name="psA", tag="bc")
                psB = pbc.tile([128, 512], F32, name="psB", tag="bc")
                nc.tensor.matmul(psA[:, 0:S], ones_row[0:1, :],
                                 rstd_b[0:1, :], start=True, stop=True)
                nc.tensor.matmul(psB[:, 0:S], ones_row[0:1, :],
                                 muneg_b[0:1, :], start=True, stop=True)
                # GpSimd cannot read PSUM: stage the broadcast rows in
                # SBUF so 2 of 8 k-chunks can normalize there while DVE
                # does the rest (bf16 temps for 2x DVE throughput)
                sbA = bcs.tile([128, S], BCD, name="sbA", tag="bcs")
                nc.scalar.copy(out=sbA[:], in_=psA[:, 0:S])
                sbB = bcs.tile([128, S], BCD, name="sbB", tag="bcs")
                nc.scalar.copy(out=sbB[:], in_=psB[:, 0:S])
                for k in range(KC):
                    tmp = lntmp.tile([128, S], BF16, name="tmp", tag="ap")
                    if k % 4 == 3:
                        nc.gpsimd.tensor_add(out=tmp[:], in0=srcbf_ap(k, b),
                                             in1=sbB[:])
                        nc.gpsimd.tensor_mul(
                            out=x_out[:, k, b * S:(b + 1) * S],
                            in0=tmp[:], in1=sbA[:])
                    else:
                        nc.vector.tensor_add(out=tmp[:], in0=srcbf_ap(k, b),
                                             in1=psB[:, 0:S])
                        nc.vector.tensor_mul(
                            out=x_out[:, k, b * S:(b + 1) * S],
                            in0=tmp[:], in1=psA[:, 0:S])

        # fp8 chunk-major activations: [128, KC, NTP]
        x1_p = top.enter_context(tc.tile_pool(name="x1", bufs=1))
        x1 = x1_p.tile([128, KC, NTP], FP8, name="x1", tag="x1")
        x2_p = top.enter_context(tc.tile_pool(name="x2", bufs=1))
        x2 = x2_p.tile([128, KC, NTP], FP8, name="x2", tag="x2")

        # residual (x + o_b)^T f32: emitted on the Scalar engine's DMA
        # queue so weight DMAs (Sync queue) are not stuck behind these
        # 4.2MB; loads overlap LN1/V/attention.
        xt_p = top.enter_context(tc.tile_pool(name="xt", bufs=KC))
        xt = []
        for k in range(KC):
            t = xt_p.tile([128, NT], F32, name="xt", tag="xt")
            nc.scalar.dma_start(
                out=t[:], in_=xT_d[k * 128:(k + 1) * 128, :])
            xt.append(t)

        # long-lived right-side pools, opened in close order (LIFO):
        ht_p = top.enter_context(
            tc.tile_pool(name="ht", bufs=KC, side="right"))
        # closed explicitly after LN2 (holds bf16 copies + squares)
        ln2prep_ph = ExitStack()
        htb_p = ln2prep_ph.enter_context(
            tc.tile_pool(name="htb", bufs=KC, side="right"))
        sq2_p = ln2prep_ph.enter_context(
            tc.tile_pool(name="sq2", bufs=KC * B_LOC, side="right"))

        with tc.tile_pool(name="ctx2", bufs=MC_E // 2) as ctx_p:
            ctx2 = [ctx_p.tile([128, 2, NTP], FP8, tag="ctx2", name="ctx2")
                    for _ in range(MC_E // 2)]

            # ============= LN1 / V / QK+attention ====================
            with tc.tile_pool(name="vpool", bufs=2 * B_LOC + 1) as v_p, \
                    tc.tile_pool(name="vt4p", bufs=1) as vt4_p:
                with ExitStack() as ln1_ph:
                    # V weights first on the Sync DMA queue; xtb goes on
                    # the Vector queue so both stream concurrently
                    vw_p = ln1_ph.enter_context(
                        tc.tile_pool(name="vw", bufs=KC // 2))
                    vw_sb = []
                    for kk in range(KC // 2):
                        vwk = vw_p.tile([128, 2, E], FP8, name="vwk",
                                        tag="vwk")
                        nc.sync.dma_start(out=vwk[:], in_=vw_d[kk, :, :, :])
                        vw_sb.append(vwk)
                    xtb_p = ln1_ph.enter_context(
                        tc.tile_pool(name="xtb", bufs=KC * B_LOC))
                    xtb = {}
                    for b in range(B_LOC):
                        for k in range(KC):
                            tb = xtb_p.tile([128, S], BF16, name="xtb",
                                            tag="xtb")
                            nc.gpsimd.dma_start(
                                out=tb[:],
                                in_=xTb_d[k * 128:(k + 1) * 128,
                                          b * S:(b + 1) * S])
                            xtb[(k, b)] = tb
                    emit_ln(ln1_ph,
                            lambda k, b: xtb[(k, b)][:],
                            lambda k, b: xtb[(k, b)][:],
                            x1, "1")

                    # ============= V projection (fp8 DoubleRow) ==========
                    v_tiles = {}
                    with ExitStack() as ph:
                        ppv = ph.enter_context(
                            tc.tile_pool(name="ppv", bufs=2, space="PSUM"))
                        for b in range(B_LOC):
                            for jc, (j0, jcs) in enumerate(JC[:2]):
                                ps = ppv.tile([128, 2, 512], F32,
                                              name="vps", tag="vps")
                                # n-outer: a start=True clears has_written
                                # for the whole bank, so only one
                                # accumulation chain may be live per bank
                                for n, (n0, nn) in enumerate(VCH):
                                    for kk in range(KC // 2):
                                        nc.tensor.matmul(
                                            ps[0:jcs, n0 // 512,
                                               n0 % 512:n0 % 512 + nn],
                                            x1[:, 2 * kk:2 * kk + 2,
                                               b * S + j0:b * S + j0 + jcs],
                                            vw_sb[kk][:, :, n0:n0 + nn],
                                            start=(kk == 0),
                                            stop=(kk == KC // 2 - 1),
                                            perf_mode=DR)
                                # [tok, H, 128]: cols 0:64 ones, cols
                                # 64:128 V -> ctx matmul replicates the
                                # softmax sums across partitions 0:64.
                                vt = v_p.tile([128, H, 128], BF16,
                                              name="vt", tag="vt")
                                if with_vbias:
                                    nc.vector.scalar_tensor_tensor(
                                        out=vt[0:jcs, :, 64:128],
                                        in0=ps[0:jcs, :, :], scalar=sv_inv,
                                        in1=vb_sb[0:jcs, :],
                                        op0=ALU.mult, op1=ALU.add)
                                else:
                                    nc.scalar.mul(
                                        out=vt[0:jcs, :, 64:128],
                                        in_=ps[0:jcs, :, :], mul=sv_inv)
                                nc.gpsimd.memset(vt[:, :, 0:64], 1.0)
                                v_tiles[(b, jc)] = vt
                        # the 4 batches' tail token (j=256): one DR pass
                        # with a strided-column lhsT (cols 256, 513, 770,
                        # 1027) produces all 4 rows at once.
                        ps = ppv.tile([128, 2, 512], F32,
                                      name="vps_t", tag="vps")
                        for n, (n0, nn) in enumerate(VCH):
                            for kk in range(KC // 2):
                                nc.tensor.matmul(
                                    ps[0:B_LOC, n0 // 512,
                                       n0 % 512:n0 % 512 + nn],
                                    x1[:, 2 * kk:2 * kk + 2, 256:NT:S],
                                    vw_sb[kk][:, :, n0:n0 + nn],
                                    start=(kk == 0),
                                    stop=(kk == KC // 2 - 1),
                                    perf_mode=DR)
                        vt_t = v_p.tile([128, H, 128], BF16,
                                        name="vt_t", tag="vt")
                        nc.gpsimd.memset(vt_t[:, :, 0:64], 1.0)
                        # drain rows 0:4 in one op (engines need 32-aligned
                        # partition bases), then DMA rows to 32b placement
                        vt4 = vt4_p.tile([128, H, 64], BF16, name="vt4",
                                         tag="vt4")
                        if with_vbias:
                            nc.vector.scalar_tensor_tensor(
                                out=vt4[0:B_LOC, :, :],
                                in0=ps[0:B_LOC, :, :],
                                scalar=sv_inv, in1=vb_sb[0:B_LOC, :],
                                op0=ALU.mult, op1=ALU.add)
                        else:
                            nc.scalar.mul(
                                out=vt4[0:B_LOC, :, :],
                                in_=ps[0:B_LOC, :, :],
                                mul=sv_inv)
                        for b in range(B_LOC):
                            nc.sync.dma_start(
                                out=vt_t[32 * b:32 * b + 1, :, 64:128],
                                in_=vt4[b:b + 1, :, :])
                            v_tiles[(b, 2)] = vt_t

                # ========= Q/K + attention (per head-pair chunk) =====
                with ExitStack() as ph:
                    qt_p = ph.enter_context(tc.tile_pool(name="qt", bufs=2))
                    kt_p = ph.enter_context(tc.tile_pool(name="kt", bufs=2))
                    wqk_p = ph.enter_context(
                        tc.tile_pool(name="wqk", bufs=4))
                    e_p = ph.enter_context(tc.tile_pool(name="ep", bufs=9))
                    rs_p = ph.enter_context(tc.tile_pool(name="rsp", bufs=4))
                    if with_mask:
                        msk_p = ph.enter_context(
                            tc.tile_pool(name="mskp", bufs=3 * B_LOC))
                    pp2 = ph.enter_context(
                        tc.tile_pool(name="pp2", bufs=1, space="PSUM"))
                    psp = ph.enter_context(
                        tc.tile_pool(name="psp", bufs=3, space="PSUM"))
                    if with_mask:
                        msk = {}
                        for b in range(B_LOC):
                            for jc, (j0, jcs) in enumerate(JC):
                                mt = msk_p.tile([128, S], F32, name="mt",
                                                tag="mt")
                                nc.sync.dma_start(
                                    out=mt[0:jcs, :],
                                    in_=mskT_d[b, j0:j0 + jcs, :])
                                msk[(b, jc)] = mt

                    for ec in range(MC_E):
                        # ps_t serves two roles this iteration: first its
                        # banks hold the Q/K projections' third column
                        # chunk, then rows 32b get the tail-key scores.
                        ps_t = psp.tile([128, 2, 512], F32,
                                        name="ps_t", tag="sp")
                        nc.vector.memset(ps_t[:], 0.0)
                        qkt = []
                        for ip, (w_d, b_sb, opool) in enumerate((
                                (qw_d, qb_sb, qt_p),
                                (kw_d, kb_sb, kt_p))):
                            wt = wqk_p.tile([128, KC, 128], FP8,
                                            name="wqk", tag="wqk")
                            nc.sync.dma_start(out=wt[:],
                                              in_=w_d[ec, :, :, :])
                            ot = opool.tile([128, NT], BF16,
                                            name="qk", tag="qk")
                            ps = pp2.tile([128, 2, 512], F32,
                                          name="pqk", tag="pqk")
                            for c, (c0, cn) in enumerate(CH3):
                                dst = (ps[:, c, 0:cn] if c < 2
                                       else ps_t[:, ip, 0:cn])
                                for kk in range(KC // 2):
                                    nc.tensor.matmul(
                                        dst,
                                        wt[:, 2 * kk:2 * kk + 2, :],
                                        x1[:, 2 * kk:2 * kk + 2,
                                           c0:c0 + cn],
                                        start=(kk == 0),
                                        stop=(kk == KC // 2 - 1),
                                        perf_mode=DR)
                            # drains split across ACT and DVE so the
                            # single pp2 buffer frees quickly
                            drains = [(ps[:, 0, 0:CH3[0][1]],) + CH3[0],
                                      (ps[:, 1, 0:CH3[1][1]],) + CH3[1],
                                      (ps_t[:, ip, 0:CH3[2][1]],) + CH3[2]]
                            for c, (src, c0, cn) in enumerate(drains):
                                if with_qkbias:
                                    nc.vector.tensor_scalar_add(
                                        out=ot[:, c0:c0 + cn], in0=src,
                                        scalar1=b_sb[:, ec:ec + 1])
                                elif (c + ip) % 2 == 0:
                                    nc.scalar.copy(out=ot[:, c0:c0 + cn],
                                                   in_=src)
                                else:
                                    nc.vector.tensor_copy(
                                        out=ot[:, c0:c0 + cn], in_=src)
                            qkt.append(ot)
                        qte, kte = qkt

                        # tail key (j=256) for all 4 batches: packed
                        # into array col groups 32b / row groups 64*hi,
                        # one shared exp over all rows.
                        for b in range(B_LOC):
                            for hi in range(2):
                                p0 = hi * 64
                                nc.tensor.matmul(
                                    ps_t[32 * b:32 * b + 1, hi, 0:S],
                                    kte[p0:p0 + 64,
                                        b * S + 256: b * S + 257],
                                    qte[p0:p0 + 64, b * S:(b + 1) * S],
                                    start=True, stop=True,
                                    tile_position=(p0, 32 * b))
                        if with_mask:
                            for b in range(B_LOC):
                                for hi in range(2):
                                    nc.vector.tensor_add(
                                        out=ps_t[32 * b:32 * b + 1, hi, 0:S],
                                        in0=ps_t[32 * b:32 * b + 1, hi, 0:S],
                                        in1=msk[(b, 2)][0:1, :])
                        et_t = e_p.tile([128, 2, S], BF16,
                                        name="et_t", tag="et")
                        nc.scalar.activation(out=et_t[:, :, :],
                                             in_=ps_t[:, :, 0:S],
                                             func=AF.Exp, scale=sqk_inv)

                        for b in range(B_LOC):
                            ets = []
                            for jc, (j0, jcs) in enumerate(JC[:2]):
                                sp = psp.tile([128, 2, 512], F32,
                                              name="sp", tag="sp")
                                for hi in range(2):
                                    p0 = hi * 64
                                    nc.tensor.matmul(
                                        sp[0:jcs, hi, 0:S],
                                        kte[p0:p0 + 64,
                                            b * S + j0: b * S + j0 + jcs],
                                        qte[p0:p0 + 64,
                                            b * S:(b + 1) * S],
                                        start=True, stop=True)
                                if with_mask:
                                    for hi in range(2):
                                        nc.vector.tensor_add(
                                            out=sp[0:jcs, hi, 0:S],
                                            in0=sp[0:jcs, hi, 0:S],
                                            in1=msk[(b, jc)][0:jcs, :])
                                et = e_p.tile([128, 2, S], BF16,
                                              name="et", tag="et")
                                nc.scalar.activation(
                                    out=et[0:jcs, :, :],
                                    in_=sp[0:jcs, :, 0:S], func=AF.Exp,
                                    scale=sqk_inv)
                                ets.append(et)
                            cp = psp.tile([128, 2, 512], F32,
                                          name="cp", tag="sp")
                            for hi in range(2):
                                h = 2 * ec + hi
                                for jc, (j0, jcs) in enumerate(JC[:2]):
                                    nc.tensor.matmul(
                                        cp[0:128, hi, 0:S],
                                        v_tiles[(b, jc)][0:jcs, h, :],
                                        ets[jc][0:jcs, hi, :],
                                        start=(jc == 0), stop=False)
                                nc.tensor.matmul(
                                    cp[0:128, hi, 0:S],
                                    v_tiles[(b, 2)][32 * b:32 * b + 1,
                                                    h, :],
                                    et_t[32 * b:32 * b + 1, hi, :],
                                    start=False, stop=True,
                                    tile_position=(32 * b, 0))
                            rst = rs_p.tile([64, 2, S], F32,
                                            name="rst", tag="rst")
                            nc.vector.reciprocal_approx_fast(
                                out=rst[0:64, :, :],
                                in_=cp[0:64, :, 0:S])
                            for hi in range(2):
                                nc.vector.tensor_mul(
                                    out=ctx2[ec // 2][hi * 64:hi * 64 + 64,
                                                      ec % 2,
                                                      b * S:(b + 1) * S],
                                    in0=cp[64:128, hi, 0:S],
                                    in1=rst[0:64, hi, :])

            # ===== out projection (fp8 DoubleRow) + residual =====
            # LN2 prep (bf16 copy + squares) is interleaved per m-tile so
            # the LN2 stats can start the moment the projection finishes.
            ht, htb, sq2 = [], [], {}
            with ExitStack() as ph:
                wo_p = ph.enter_context(tc.tile_pool(name="wo", bufs=3))
                ppo = ph.enter_context(
                    tc.tile_pool(name="ppo", bufs=2, space="PSUM"))
                for m in range(MC_E):
                    wt = wo_p.tile([128, KC, 128], FP8, name="wo",
                                   tag="wo")
                    nc.sync.dma_start(out=wt[:], in_=ow_d[m, :, :, :])
                    ps = ppo.tile([128, 3, 512], F32, name="po",
                                  tag="po")
                    for c, (c0, cn) in enumerate(CH3):
                        for kk in range(KC // 2):
                            nc.tensor.matmul(
                                ps[:, c, 0:cn],
                                wt[:, 2 * kk:2 * kk + 2, :],
                                ctx2[kk][:, :, c0:c0 + cn],
                                start=(kk == 0), stop=(kk == KC // 2 - 1),
                                perf_mode=DR)
                    o = ht_p.tile([128, NT], F32, name="ht", tag="ht")
                    for c, (c0, cn) in enumerate(CH3):
                        nc.vector.scalar_tensor_tensor(
                            out=o[:, c0:c0 + cn],
                            in0=ps[:, c, 0:cn],
                            scalar=so_inv,
                            in1=xt[m][:, c0:c0 + cn],
                            op0=ALU.mult, op1=ALU.add)
                    ht.append(o)
                    hb = htb_p.tile([128, NT], BF16, name="htb", tag="htb")
                    nc.vector.tensor_copy(out=hb[:], in_=o[:])
                    htb.append(hb)
                    for b in range(B_LOC):
                        sq = sq2_p.tile([128, S], BF16, name="sq2",
                                        tag="sq2")
                        if b % 2 == 0:
                            nc.scalar.activation(
                                out=sq[:], in_=hb[:, b * S:(b + 1) * S],
                                func=AF.Square)
                        else:
                            nc.gpsimd.tensor_mul(
                                out=sq[:], in0=hb[:, b * S:(b + 1) * S],
                                in1=hb[:, b * S:(b + 1) * S])
                        sq2[(m, b)] = sq
        # ctx2 closed

        # ================= LN2 + MLP =====================================
        with ExitStack() as ln2_ph:
            emit_ln(ln2_ph,
                    lambda k, b: ht[k][:, b * S:(b + 1) * S],
                    lambda k, b: htb[k][:, b * S:(b + 1) * S],
                    x2, "2", sq_pre=sq2, split_apply=True)
        ln2prep_ph.close()
        # f1o reuses the SBUF freed by htb/sq2
        f1o_p = top.enter_context(
            tc.tile_pool(name="f1o", bufs=MC_I, side="right"))

        # ============= fc1 (fp8 DoubleRow) ==============================
        f1o = []
        with ExitStack() as ph:
            # fc2 weight pool opens early so the first fc2 weights prefetch
            # during fc1
            wf2_p = ph.enter_context(tc.tile_pool(name="wf2", bufs=2))
            f2w_t = {}
            with ExitStack() as ph1:
                wf1_p = ph1.enter_context(tc.tile_pool(name="wf1", bufs=4))
                ppf1 = ph1.enter_context(
                    tc.tile_pool(name="ppf1", bufs=2, space="PSUM"))
                for m in range(MC_I):
                    wt = wf1_p.tile([128, KC, 128], FP8, name="wf1",
                                    tag="wf1")
                    nc.sync.dma_start(out=wt[:], in_=f1w_d[m, :, :, :])
                    o = f1o_p.tile([128, NT], BF16, name="f1o", tag="f1o")
                    ps = ppf1.tile([128, 3, 512], F32, name="pf1", tag="pf1")
                    for c, (c0, cn) in enumerate(CH3):
                        for kk in range(KC // 2):
                            nc.tensor.matmul(
                                ps[:, c, 0:cn],
                                wt[:, 2 * kk:2 * kk + 2, :],
                                x2[:, 2 * kk:2 * kk + 2, c0:c0 + cn],
                                start=(kk == 0), stop=(kk == KC // 2 - 1),
                                perf_mode=DR)
                    # two ACT ops instead of three: [c0|c1] fused via a 3D
                    # AP (equal 343-col chunks), keeping gelu off fc1's
                    # critical path (3 ops made fc1 ACT-bound)
                    nc.scalar.activation(
                        out=o[:, 0:686], in_=ps[:, 0:2, 0:343],
                        func=AF.Gelu_apprx_tanh, scale=sf1_inv,
                        bias=f1b_sb[:, m:m + 1])
                    nc.scalar.activation(
                        out=o[:, 686:1028], in_=ps[:, 2, 0:342],
                        func=AF.Gelu_apprx_tanh, scale=sf1_inv,
                        bias=f1b_sb[:, m:m + 1])
                    f1o.append(o)
                    if m == MC_I // 2:
                        # kick off fc2 weight DMAs mid-fc1 so the fc2 loop
                        # starts hot
                        for mm in range(2):
                            wt2 = wf2_p.tile([128, MC_I, 128], BF16,
                                             name="wf2", tag="wf2")
                            nc.sync.dma_start(out=wt2[:],
                                              in_=f2w_d[mm, :, :, :])
                            f2w_t[mm] = wt2

            # ============= fc2 (bf16, k-outer, 343-col chunks) ==========
            with ExitStack() as ph2:
                ppf2 = ph2.enter_context(
                    tc.tile_pool(name="ppf2", bufs=2, space="PSUM"))
                out_p = ph2.enter_context(tc.tile_pool(name="outp", bufs=3))
                for m in range(MC_E):
                    if m not in f2w_t:
                        wt2 = wf2_p.tile([128, MC_I, 128], BF16,
                                         name="wf2", tag="wf2")
                        nc.sync.dma_start(out=wt2[:], in_=f2w_d[m, :, :, :])
                        f2w_t[m] = wt2
                    wt = f2w_t.pop(m)
                    ps = ppf2.tile([128, 3, 512], F32, name="pf2", tag="pf2")
                    for k in range(MC_I):
                        for c, (c0, cn) in enumerate(CH3):
                            nc.tensor.matmul(
                                ps[:, c, 0:cn], wt[:, k, :],
                                f1o[k][:, c0:c0 + cn],
                                start=(k == 0), stop=(k == MC_I - 1))
                    o = out_p.tile([128, NT], F32, name="oo", tag="oo")
                    for c, (c0, cn) in enumerate(CH3):
                        nc.vector.scalar_tensor_tensor(
                            out=o[:, c0:c0 + cn], in0=ps[:, c, 0:cn],
                            scalar=f2b_sb[:, m:m + 1],
                            in1=ht[m][:, c0:c0 + cn],
                            op0=ALU.add, op1=ALU.add)
                    nc.sync.dma_start(out=outT_d[m * 128:(m + 1) * 128, :],
                                      in_=o[:])

    nc.compile()
    return nc


def _pow2_scale(W):
    """Largest power-of-two s with max|W|*s <= F8MAX."""
    m = float(np.abs(W).max())
    if m == 0.0:
        return 1.0
    return 2.0 ** np.floor(np.log2(F8MAX / m))


def _pack_lhsT8(W, s):
    """W [M, K] (out, in) -> [M/128, 128, K/128, 128] fp8 with
    [m, p, k, j] = W[m*128+j, k*128+p] * s (lhsT tiles, partition = K)."""
    W = np.asarray(W, np.float32) * s
    M, K = W.shape
    A = W.reshape(M // 128, 128, K // 128, 128)
    A = np.ascontiguousarray(A.transpose(0, 3, 2, 1))
    return np.clip(A, -F8MAX, F8MAX).astype(ml_dtypes.float8_e4m3)


def _pack_lhsT(W):
    """W [M, K] -> [M/128, 128, K/128, 128] bf16 lhsT tiles."""
    W = np.asarray(W, np.float32)
    M, K = W.shape
    A = W.reshape(M // 128, 128, K // 128, 128)
    return np.ascontiguousarray(A.transpose(0, 3, 2, 1)).astype(ml_dtypes.bfloat16)


def _pack_pbias(b):
    """b [M] -> [128, M/128] f32 per-partition bias columns."""
    return np.ascontiguousarray(np.asarray(b, np.float32).reshape(-1, 128).T)


def kernel(hidden_states, attention_mask, causal_attention_mask,
           ln1_w, ln1_b, q_w, q_b, k_w, k_b, v_w, v_b, o_w, o_b,
           ln2_w, ln2_b, fc1_w, fc1_b, fc2_w, fc2_b):
    global LAST_EXEC_NS
    from concourse.bass_utils import run_bass_kernel_spmd

    hs = np.asarray(hidden_states, np.float32)
    msk = (np.asarray(attention_mask, np.float32)
           + np.asarray(causal_attention_mask, np.float32))
    with_mask = bool(np.any(msk))

    ln1_w = np.asarray(ln1_w, np.float32); ln1_b = np.asarray(ln1_b, np.float32)
    ln2_w = np.asarray(ln2_w, np.float32); ln2_b = np.asarray(ln2_b, np.float32)
    q_w = np.asarray(q_w, np.float32); q_b = np.asarray(q_b, np.float32)
    k_w = np.asarray(k_w, np.float32); k_b = np.asarray(k_b, np.float32)
    v_w = np.asarray(v_w, np.float32); v_b = np.asarray(v_b, np.float32)
    o_w = np.asarray(o_w, np.float32); o_b = np.asarray(o_b, np.float32)
    fc1_w = np.asarray(fc1_w, np.float32); fc1_b = np.asarray(fc1_b, np.float32)
    fc2_w = np.asarray(fc2_w, np.float32); fc2_b = np.asarray(fc2_b, np.float32)

    scale = D ** -0.5
    # fold LN1 scale/bias into Q/K/V, and the softmax scale into Q
    qw_eff = (q_w * ln1_w[None, :]) * scale
    qb_eff = (q_b + q_w @ ln1_b) * scale
    kw_eff = k_w * ln1_w[None, :]
    kb_eff = k_b + k_w @ ln1_b
    vw_eff = v_w * ln1_w[None, :]
    vb_eff = v_b + v_w @ ln1_b
    # fold LN2 into fc1
    f1w_eff = fc1_w * ln2_w[None, :]
    f1b_eff = fc1_b + fc1_w @ ln2_b

    # fp8 power-of-two scales
    s_q = _pow2_scale(qw_eff)
    s_k = _pow2_scale(kw_eff)
    s_v = _pow2_scale(vw_eff)
    s_o = _pow2_scale(o_w)
    s_f1 = _pow2_scale(f1w_eff)

    vw8 = np.clip(vw_eff.T * s_v, -F8MAX, F8MAX).astype(
        ml_dtypes.float8_e4m3).reshape(KC // 2, 2, 128, E).transpose(0, 2, 1, 3)

    base = {
        "qw": _pack_lhsT8(qw_eff, s_q),
        "kw": _pack_lhsT8(kw_eff, s_k),
        "vw": np.ascontiguousarray(vw8),
        "ow": _pack_lhsT8(o_w, s_o),
        "f1w": _pack_lhsT8(f1w_eff, s_f1),
        "f2w": _pack_lhsT(fc2_w),
        "qb": _pack_pbias(qb_eff * s_q),
        "kb": _pack_pbias(kb_eff * s_k),
        "vb": np.ascontiguousarray(vb_eff[None, :].astype(np.float32)),
        "f1b": _pack_pbias(f1b_eff),
        "f2b": _pack_pbias(fc2_b),
    }

    with_vbias = bool(np.any(vb_eff))
    with_qkbias = bool(np.any(qb_eff)) or bool(np.any(kb_eff))
    key = (with_mask, with_vbias, with_qkbias, s_q, s_k, s_v, s_o, s_f1)
    if key not in _cache:
        _cache[key] = _build(with_mask, with_vbias, with_qkbias,
                             1.0 / s_v, 1.0 / s_o, 1.0 / (s_q * s_k),
                             1.0 / s_f1)
    nc = _cache[key]

    # o_b folded into the residual
    res = hs + o_b[None, None, :]

    in_maps = []
    for c in range(N_CORES):
        x = hs[c * B_LOC:(c + 1) * B_LOC].reshape(NT, E).T
        r = res[c * B_LOC:(c + 1) * B_LOC].reshape(NT, E).T
        m = dict(base)
        m["xT"] = np.ascontiguousarray(r)
        m["xTb"] = np.ascontiguousarray(x).astype(ml_dtypes.bfloat16)
        if with_mask:
            m["mskT"] = np.ascontiguousarray(
                msk[c * B_LOC:(c + 1) * B_LOC, 0].transpose(0, 2, 1)
                * (s_q * s_k))
        in_maps.append(m)

    res_k = run_bass_kernel_spmd(nc, in_maps, core_ids=list(range(N_CORES)),
                                 trace=TRACE)
    LAST_EXEC_NS = res_k.exec_time_ns

    outs = []
    for c in range(N_CORES):
        oT = res_k.results[c]["outT"]          # [E, NT] f32
        outs.append(np.ascontiguousarray(oT.T).reshape(B_LOC, S, E))
    return np.concatenate(outs, axis=0)
